# revision 41
# baseline (speedup 1.0000x reference)
"""Multi-head attention (B=4, S=2048, D=1024, H=16) on 8 TRN2 NeuronCores.

Sharding: 2D - batch 4-way x head-group 2-way. Core c handles batch b = c//2
and head group j = c%2 (8 heads, model-dim slice j*512:(j+1)*512 of the QKV
projections / rows j*512:(j+1)*512 of Wo). Each core computes a partial
[S, D] output (row-sharded Wo => partial sums); the host adds the two
partials per batch and the output bias.

Engine model (per core): ACT exp work = 8*2048^2 = 33.5M elem @ ~1.09ns
 => ~285us; PE matmul stream = attention 164us + projections 109us. The
schedule keeps ACT saturated while dribbling projection matmuls into the
per-step PE slack, in work-deadline order.

Device-side layout notes:
 - Activations kept transposed ([feature, seq]); host pre-tiles all inputs
   into [block][128, 512] contiguous chunks so every DMA is a single dense
   128KB+ transfer.
 - Attention uses the S^T layout: S^T[t, s] tiles come straight from
   lhsT=K^T, rhs=Q^T matmuls (two heads row-packed via tile_position);
   softmax-exp runs on ACT (scale=1/8 fused); V carries an appended ones
   column per head so the PV matmul also produces softmax denominators.
 - Prelude: PE warm-up matmuls + ACT table preload run during the DMA dead
   time; attention starts as soon as K(m0,s0)/Q(0,s0) land (~19us), with
   V/K/Q chains dribbled just-in-time behind the DMA arrival order.
 - Output is written as bf16 [ko][q] blocks; host reassembles, adds the two
   core partials in fp32 and the output bias.
 - The reference's "causal mask" adds log(1 + 1e-6) ~ 1e-6 to allowed
   logits - far below bf16 noise - so it is deliberately not applied.
"""

import os
import sys
import types

sys.path.insert(0, "/opt/trn_rl_repo")

import numpy as np
import ml_dtypes

B, S, D, H = 4, 2048, 1024, 16
PD = D // H          # 64 head dim
P = 128              # partitions
NCORES = 8
DP = 512             # d' (feature) slice per core = 8 heads
KO = D // P          # 8 contraction tiles for QKV projections
MT = DP // P         # 4 feature tiles (= head pairs)
NT = S // P          # 16 key/time tiles
NSB = 4              # s blocks
SBW = S // NSB       # 512 block width
HL = 8               # local heads
VW = 65              # V columns per head incl. ones column

BF16 = ml_dtypes.bfloat16

_NC = None
LAST_RUN = {}


def _install_ntff_shim():
    """bass_utils' axon trace path imports antenv.axon_hooks, which this
    image lacks; register the hook module manually so BASS_TRACE works."""
    if "antenv.axon_hooks" in sys.modules:
        return
    try:
        import trn_agent_boot.trn_boot as tb

        hook = tb._ntff_profile_via_ctypes("/opt/axon/libaxon_pjrt.so")
    except Exception:
        hook = None
    m = types.ModuleType("antenv.axon_hooks")
    m.get_axon_ntff_profile_hook = lambda: hook
    m.set_axon_ntff_profile_hook = lambda h: None
    sys.modules["antenv.axon_hooks"] = m


def _build():
    import concourse.tile as tile
    from concourse import bacc, mybir

    CDT = mybir.dt.bfloat16
    F32 = mybir.dt.float32
    EXP = mybir.ActivationFunctionType.Exp
    ADD = mybir.AluOpType.add
    MUL = mybir.AluOpType.mult

    nc = bacc.Bacc("TRN2", target_bir_lowering=False, debug=False)

    # All activations pre-tiled by the host: [ko, q, 128, 512] contiguous.
    # block-major layouts: one whole-block DMA each, 4-16KB partition lines
    qT_d = nc.dram_tensor("qT", [NSB, P, KO, SBW], CDT, kind="ExternalInput").ap()
    kT_d = nc.dram_tensor(
        "kT", [NSB // 2, P, KO, 2 * SBW], CDT, kind="ExternalInput"
    ).ap()
    vT_d = nc.dram_tensor("vT", [NSB, P, KO, SBW], CDT, kind="ExternalInput").ap()
    # wq/wk m-sliced: [m][128, ko, 128] contiguous per m.
    wq_d = nc.dram_tensor("wq", [MT, P, KO, P], CDT, kind="ExternalInput").ap()
    wk_d = nc.dram_tensor("wk", [MT, P, KO, P], CDT, kind="ExternalInput").ap()
    wv_d = nc.dram_tensor("wv", [P, KO, DP], CDT, kind="ExternalInput").ap()
    wo_d = nc.dram_tensor("wo", [P, MT, D], CDT, kind="ExternalInput").ap()
    bq_d = nc.dram_tensor("bq", [P, MT], F32, kind="ExternalInput").ap()
    bk_d = nc.dram_tensor("bk", [P, MT], F32, kind="ExternalInput").ap()
    bv_d = nc.dram_tensor("bv", [1, DP], CDT, kind="ExternalInput").ap()
    yT_d = nc.dram_tensor("yT", [KO, NSB, P, SBW], CDT, kind="ExternalOutput").ap()

    with tile.TileContext(nc) as tc:
        import contextlib

        with contextlib.ExitStack() as ctx:
            wkp = ctx.enter_context(tc.tile_pool(name="wk", bufs=4))
            wqp = ctx.enter_context(tc.tile_pool(name="wq", bufs=4))
            wvop = ctx.enter_context(tc.tile_pool(name="wvo", bufs=1))
            biasp = ctx.enter_context(tc.tile_pool(name="bias", bufs=1))
            kTp = ctx.enter_context(tc.tile_pool(name="kTp", bufs=2))
            vTp = ctx.enter_context(tc.tile_pool(name="vTp", bufs=2))
            qTp = ctx.enter_context(tc.tile_pool(name="qTp", bufs=2))
            actp = ctx.enter_context(tc.tile_pool(name="acts", bufs=1))
            expp = ctx.enter_context(tc.tile_pool(name="exps", bufs=5))
            op = ctx.enter_context(tc.tile_pool(name="otile", bufs=6))
            bcp = ctx.enter_context(tc.tile_pool(name="bcast", bufs=1))
            yp = ctx.enter_context(tc.tile_pool(name="ystage", bufs=2))
            # PSUM budget (8 banks): S^T pair tiles 2x2 + proj/out 2 + psO 2
            ps_pair = ctx.enter_context(
                tc.tile_pool(name="ps_pair", bufs=2, space="PSUM")
            )
            ps_mm = ctx.enter_context(tc.tile_pool(name="ps_mm", bufs=2, space="PSUM"))
            ps_o = ctx.enter_context(tc.tile_pool(name="ps_o", bufs=2, space="PSUM"))

            # ---- DMA emission order is the schedule's backbone --------------
            # (~250-320 GB/s effective; every block below is a single dense
            # contiguous transfer, ordered by first use.)
            bv_sb = biasp.tile([P, DP], CDT, tag="bv")
            nc.sync.dma_start(bv_sb[:], bv_d[0:1, :].to_broadcast((P, DP)))
            bq_sb = biasp.tile([P, MT], F32, tag="bq")
            nc.sync.dma_start(bq_sb[:], bq_d[:])
            bk_sb = biasp.tile([P, MT], F32, tag="bk")
            nc.sync.dma_start(bk_sb[:], bk_d[:])

            # ---- engine warm-up (runs during the DMA dead time) -------------
            # ~10us of dummy matmuls keeps the PE HAM un-throttled (2.4GHz)
            # by the time real chains start; a dummy exp preloads the ACT
            # spline table set (~2.7us) off the critical path.
            es = expp.tile([P, 2 * SBW], CDT, tag="e", name="e_warm")
            nc.vector.memset(es[0:1, 0:32], 0.0)
            nc.scalar.activation(es[0:1, 16:32], es[0:1, 0:16], EXP)
            for i in range(48):
                ps_w = ps_mm.tile([P, SBW], F32, tag="ps", name="ps_warm")
                nc.tensor.matmul(
                    ps_w[:], bv_sb[:, 0:P], bv_sb[:, 0:DP], start=True, stop=True
                )

            wk_sb = {}
            wq_sb = {}

            def load_w(pool, w_d, dst, m, tag):
                t_sb = pool.tile([P, KO, P], CDT, tag=f"{tag}{m}")
                nc.sync.dma_start(t_sb[:], w_d[m])
                dst[m] = t_sb

            vtile = {}
            ktile = {}
            qtile = {}

            def load_block(t_dram, dst, q, tag):
                # whole [128, KO*512] block in ONE DMA (8KB partition lines)
                pool = {"vT": vTp, "qT": qTp}[tag]
                t_sb = pool.tile([P, KO, SBW], CDT, tag=tag, name=f"{tag}_{q}")
                nc.sync.dma_start(t_sb[:], t_dram[q])
                for ko in range(KO):
                    dst[(ko, q)] = t_sb[:, ko, :]

            def load_block2(t_dram, dst, q, tag):
                # kT (q, q+1) pair in ONE DMA (16KB partition lines)
                t_sb = kTp.tile(
                    [P, KO, 2 * SBW], CDT, tag=tag, name=f"{tag}_{q}"
                )
                nc.sync.dma_start(t_sb[:], t_dram[q // 2])
                for ko in range(KO):
                    dst[(ko, q)] = t_sb[:, ko, 0:SBW]
                    dst[(ko, q + 1)] = t_sb[:, ko, SBW : 2 * SBW]

            load_w(wkp, wk_d, wk_sb, 0, "wk")
            load_block2(kT_d, ktile, 0, "kT")
            load_w(wqp, wq_d, wq_sb, 0, "wq")
            load_block(qT_d, qtile, 0, "qT")
            wv_sb = wvop.tile([P, KO, DP], CDT, tag="wvo", name="wv_sb")
            nc.sync.dma_start(wv_sb[:], wv_d[:])
            load_block(vT_d, vtile, 0, "vT")
            load_block(vT_d, vtile, 1, "vT")
            load_block2(kT_d, ktile, 2, "kT")
            for m in (1, 2, 3):
                load_w(wkp, wk_d, wk_sb, m, "wk")
            for m in (1, 2, 3):
                load_w(wqp, wq_d, wq_sb, m, "wq")
            load_block(vT_d, vtile, 2, "vT")
            load_block(vT_d, vtile, 3, "vT")
            load_block(qT_d, qtile, 1, "qT")

            def q_block(ko, sbk):
                if (ko, sbk) not in qtile:
                    load_block(qT_d, qtile, sbk, "qT")
                return qtile[(ko, sbk)]

            # ---- projection building blocks --------------------------------
            Vp = actp.tile([P, NT, HL * VW], CDT, tag="Vp")
            nc.vector.memset(
                Vp[:].rearrange("p t (h c) -> p t h c", c=VW)[:, :, :, PD : PD + 1],
                1.0,
            )

            KpT = actp.tile([P, MT, S], CDT, tag="KpT")
            QpT = actp.tile([P, MT, S], CDT, tag="QpT")

            def emit_v_chain(t):
                with nc.named_scope("proj_v"):
                    ps = ps_mm.tile([P, DP], F32, tag="ps", name="ps_v")
                    for ko in range(KO):
                        nc.tensor.matmul(
                            ps[:],
                            vtile[(ko, t // 4)][:, (t % 4) * P : (t % 4 + 1) * P],
                            wv_sb[:, ko, :],
                            start=(ko == 0),
                            stop=(ko == KO - 1),
                        )
                    nc.vector.tensor_tensor(
                        Vp[:, t, :].rearrange("p (h c) -> p h c", c=VW)[:, :, 0:PD],
                        ps[:].rearrange("p (h c) -> p h c", c=PD),
                        bv_sb[:].rearrange("p (h c) -> p h c", c=PD),
                        ADD,
                    )

            def emit_k_chain(m, sbk):
                with nc.named_scope("proj_k"):
                    ps = ps_mm.tile([P, SBW], F32, tag="ps", name="ps_k")
                    for ko in range(KO):
                        nc.tensor.matmul(
                            ps[:],
                            wk_sb[m][:, ko, :],
                            ktile[(ko, sbk)][:],
                            start=(ko == 0),
                            stop=(ko == KO - 1),
                        )
                    nc.vector.tensor_scalar_add(
                        KpT[:, m, sbk * SBW : (sbk + 1) * SBW],
                        ps[:],
                        bk_sb[:, m : m + 1],
                    )

            def emit_q_chain(m, sbk):
                with nc.named_scope("proj_q"):
                    ps = ps_mm.tile([P, SBW], F32, tag="ps", name="ps_q")
                    for ko in range(KO):
                        nc.tensor.matmul(
                            ps[:],
                            wq_sb[m][:, ko, :],
                            q_block(ko, sbk)[:],
                            start=(ko == 0),
                            stop=(ko == KO - 1),
                        )
                    nc.vector.tensor_scalar_add(
                        QpT[:, m, sbk * SBW : (sbk + 1) * SBW],
                        ps[:],
                        bq_sb[:, m : m + 1],
                    )

            # ---- work-item queues (dribbled between attention steps) -------
            work_q = []   # urgent: V/K/Q projection chains, deadline-ordered
            lazy_q = []   # output-projection chains (no hard deadline)

            def push_v_chain(t):
                chain = {}

                def mk(ko, t=t, chain=chain):
                    if "ps" not in chain:
                        chain["ps"] = ps_mm.tile([P, DP], F32, tag="ps", name="ps_v")
                    ps = chain["ps"]
                    nc.tensor.matmul(
                        ps[:],
                        vtile[(ko, t // 4)][:, (t % 4) * P : (t % 4 + 1) * P],
                        wv_sb[:, ko, :],
                        start=(ko == 0),
                        stop=(ko == KO - 1),
                    )
                    if ko == KO - 1:
                        nc.vector.tensor_tensor(
                            Vp[:, t, :].rearrange("p (h c) -> p h c", c=VW)[
                                :, :, 0:PD
                            ],
                            ps[:].rearrange("p (h c) -> p h c", c=PD),
                            bv_sb[:].rearrange("p (h c) -> p h c", c=PD),
                            ADD,
                        )
                for ko in range(KO):
                    work_q.append(lambda ko=ko, mk=mk: mk(ko))

            def push_k_chain(m, sbk):
                chain = {}

                def mk(ko, m=m, sbk=sbk, chain=chain):
                    if "ps" not in chain:
                        chain["ps"] = ps_mm.tile([P, SBW], F32, tag="ps", name="ps_k")
                    ps = chain["ps"]
                    nc.tensor.matmul(
                        ps[:],
                        wk_sb[m][:, ko, :],
                        ktile[(ko, sbk)][:],
                        start=(ko == 0),
                        stop=(ko == KO - 1),
                    )
                    if ko == KO - 1:
                        nc.vector.tensor_scalar_add(
                            KpT[:, m, sbk * SBW : (sbk + 1) * SBW],
                            ps[:],
                            bk_sb[:, m : m + 1],
                        )
                for ko in range(KO):
                    work_q.append(lambda ko=ko, mk=mk: mk(ko))

            def push_q_chain(m, sbk):
                chain = {}

                def mk(ko, m=m, sbk=sbk, chain=chain):
                    if "ps" not in chain:
                        chain["ps"] = ps_mm.tile([P, SBW], F32, tag="ps", name="ps_q")
                    ps = chain["ps"]
                    nc.tensor.matmul(
                        ps[:],
                        wq_sb[m][:, ko, :],
                        q_block(ko, sbk)[:],
                        start=(ko == 0),
                        stop=(ko == KO - 1),
                    )
                    if ko == KO - 1:
                        nc.vector.tensor_scalar_add(
                            QpT[:, m, sbk * SBW : (sbk + 1) * SBW],
                            ps[:],
                            bq_sb[:, m : m + 1],
                        )
                for ko in range(KO):
                    work_q.append(lambda ko=ko, mk=mk: mk(ko))

            def push_y_chains(sb, otiles):
                for n in range(KO):
                    chain = {}

                    def mk(hp, n=n, sb=sb, otiles=otiles, chain=chain):
                        if "ps" not in chain:
                            chain["ps"] = ps_mm.tile(
                                [P, SBW], F32, tag="ps", name="ps_y"
                            )
                        psY = chain["ps"]
                        nc.tensor.matmul(
                            psY[:],
                            wo_sb[:, hp, n * P : (n + 1) * P],
                            otiles[hp][:],
                            start=(hp == 0),
                            stop=(hp == MT - 1),
                        )
                        if hp == MT - 1:
                            y_sb = yp.tile([P, SBW], CDT, tag="y")
                            nc.vector.tensor_copy(y_sb[:], psY[:])
                            nc.sync.dma_start(yT_d[n, sb], y_sb[:])
                    for hp in range(MT):
                        lazy_q.append(lambda hp=hp, mk=mk: mk(hp))

            def pull(n, lazy_ok=True):
                for _ in range(n):
                    if work_q:
                        work_q.pop(0)()
                    elif lazy_ok and lazy_q:
                        lazy_q.pop(0)()

            # ---- normalize + attention (flat (hp, t) pipeline per s-block) --
            def emit_norm(hp, psO, otiles):
                with nc.named_scope("norm"):
                    # den copies first: the broadcast+recip chain is the
                    # critical path to o_t (y-chain pulls wait on it)
                    bcd = bcp.tile([P, 2 * SBW], F32, tag="bcd")
                    nc.vector.tensor_copy(bcd[0:1, 0:SBW], psO[0][PD : PD + 1, :])
                    nc.vector.tensor_copy(
                        bcd[0:1, SBW : 2 * SBW], psO[1][PD : PD + 1, :]
                    )
                    nc.gpsimd.partition_broadcast(bcd[:], bcd[0:1, :])
                    nc.vector.reciprocal_approx_fast(bcd[:], bcd[:])
                    o_t = op.tile([P, SBW], CDT, tag="o")
                    nc.vector.tensor_tensor(
                        o_t[0:PD, :], psO[0][0:PD, :], bcd[0:PD, 0:SBW], MUL
                    )
                    nc.vector.tensor_tensor(
                        o_t[PD:P, :], psO[1][0:PD, :], bcd[PD:P, SBW : 2 * SBW], MUL
                    )
                    otiles.append(o_t)

            def emit_attn_sb(sb, otiles, rate):
                steps = [(hp, t) for hp in range(MT) for t in range(NT)]
                psO = {}
                psS = {}

                def s_mm(hp, t):
                    psT = ps_pair.tile([P, 2 * SBW], F32, tag="psT", name="psT")
                    psS[(hp, t)] = psT
                    for u in range(2):
                        nc.tensor.matmul(
                            psT[:, u * SBW : (u + 1) * SBW],
                            KpT[u * PD : (u + 1) * PD, hp, t * P : (t + 1) * P],
                            QpT[
                                u * PD : (u + 1) * PD,
                                hp,
                                sb * SBW : (sb + 1) * SBW,
                            ],
                            start=True,
                            stop=True,
                            tile_position=(u * PD, 0),
                        )

                with nc.named_scope("attn"):
                    s_mm(*steps[0])
                    for i, (hp, t) in enumerate(steps):
                        n_pull, lazy_ok = rate(i)
                        pull(n_pull, lazy_ok)
                        if t == 0:
                            psO[hp] = [
                                ps_o.tile([VW, SBW], F32, tag="psO", name=f"psO{u}")
                                for u in range(2)
                            ]
                        if i + 1 < len(steps):
                            s_mm(*steps[i + 1])
                        psT = psS.pop((hp, t))
                        e = expp.tile([P, 2 * SBW], CDT, tag="e")
                        nc.scalar.activation(
                            e[:], psT[:], EXP, scale=1.0 / np.sqrt(PD)
                        )
                        for u in range(2):
                            h = 2 * hp + u
                            nc.tensor.matmul(
                                psO[hp][u][:],
                                Vp[:, t, h * VW : (h + 1) * VW],
                                e[:, u * SBW : (u + 1) * SBW],
                                start=(t == 0),
                                stop=(t == NT - 1),
                            )
                        if t == NT - 1:
                            emit_norm(hp, psO.pop(hp), otiles)

            # ---- schedule ---------------------------------------------------
            # Inline prelude: only what gates the very first attention steps.
            emit_k_chain(0, 0)
            emit_q_chain(0, 0)
            emit_v_chain(0)
            # Everything else dribbles in DMA-arrival / deadline order.
            for t in (1, 2, 3):
                push_v_chain(t)
            push_k_chain(0, 1)
            for t in (4, 5, 6, 7):
                push_v_chain(t)
            push_k_chain(0, 2)
            for t in (8, 9, 10, 11):
                push_v_chain(t)
            push_k_chain(0, 3)
            for t in (12, 13, 14, 15):
                push_v_chain(t)
            for m in (1, 2, 3):
                push_k_chain(m, 0)
                push_q_chain(m, 0)
                for sbk in (1, 2, 3):
                    push_k_chain(m, sbk)

            # wo shares wv's pool slot; the DMA is emitted here but lands
            # late in the DMA order (v chains all emitted by then on PE).
            wo_sb = wvop.tile([P, MT, D], CDT, tag="wvo", name="wo_sb")
            nc.sync.dma_start(wo_sb[:], wo_d[:])

            def mk_rate(sb):
                def rate(i):
                    hp_step = i % NT
                    if sb == 0:
                        # deadline-matched dribble: V+K m0 in hp0, K m1-3
                        # and Q chains spread over hp1-3 (~2.5/step)
                        n = 10 if i < 16 else (3 if i % 2 == 0 else 2)
                        return (n, False)
                    # avoid stalling PE on the previous head-pair's norm
                    # chain (~5us): no lazy (y-chain) pulls until it is done.
                    return (2, hp_step >= 7)
                return rate

            otiles_by_sb = {sb: [] for sb in range(NSB)}
            for sb in range(NSB):
                if sb + 2 < NSB and (0, sb + 2) not in qtile:
                    # qT block sb+2 lands during sb, consumed during sb+1
                    load_block(qT_d, qtile, sb + 2, "qT")
                if sb + 1 < NSB:
                    for m in range(MT):
                        push_q_chain(m, sb + 1)
                emit_attn_sb(sb, otiles_by_sb[sb], mk_rate(sb))
                push_y_chains(sb, otiles_by_sb[sb])
            # keep the PE HAM warm through the final norm chain so the
            # drained output-projection matmuls run at 2.4GHz
            for i in range(8):
                ps_w = ps_pair.tile([P, SBW], F32, tag="psT", name="ps_tailwarm")
                nc.tensor.matmul(
                    ps_w[:, 0:P], bv_sb[:, 0:P], bv_sb[:, 0:P],
                    start=True, stop=True,
                )
            # drain whatever is left (final output projections)
            pull(len(work_q) + len(lazy_q))

    nc.compile()
    return nc


def _get_nc():
    global _NC
    if _NC is None:
        _install_ntff_shim()
        _NC = _build()
    return _NC


def _tile_blocks(xT):
    """[D, S] -> [KO, NSB, P, SBW] contiguous blocks."""
    return np.ascontiguousarray(
        xT.reshape(KO, P, NSB, SBW).transpose(0, 2, 1, 3)
    )


def make_in_maps(q, k, v, Wq, bq, Wk, bk, Wv, bv, Wo):
    """Shard + lay out the full inputs into the 8 per-core input maps."""
    in_maps = []
    for c in range(NCORES):
        b, j = divmod(c, 2)
        d0 = j * DP
        qT = np.ascontiguousarray(q[b].T).astype(BF16)
        kT = np.ascontiguousarray(k[b].T).astype(BF16)
        vT = np.ascontiguousarray(v[b].T).astype(BF16)
        wq = Wq[:, d0 : d0 + DP].astype(BF16)  # [D, DP]
        wk = Wk[:, d0 : d0 + DP].astype(BF16)
        wv = Wv[:, d0 : d0 + DP].astype(BF16)
        wo = Wo[d0 : d0 + DP, :].astype(BF16)  # [DP, D]
        # wq/wk m-sliced [MT, P, KO, P]: slice m covers out-cols m*128..,
        # [contraction-part p, ko, out-col] per slice.
        wq4 = np.ascontiguousarray(
            wq.reshape(KO, P, MT, P).transpose(2, 1, 0, 3)
        )
        wk4 = np.ascontiguousarray(
            wk.reshape(KO, P, MT, P).transpose(2, 1, 0, 3)
        )
        # wv [P, KO, DP]: [contraction-part, ko, out-col]
        wv3 = np.ascontiguousarray(wv.reshape(KO, P, DP).transpose(1, 0, 2))
        # wo [P, MT, D]: [contraction-part within hp, hp, out-col]
        wo3 = np.ascontiguousarray(wo.reshape(MT, P, D).transpose(1, 0, 2))
        in_maps.append(
            {
                "qT": np.ascontiguousarray(
                    qT.reshape(KO, P, NSB, SBW).transpose(2, 1, 0, 3)
                ),
                "kT": np.ascontiguousarray(
                    kT.reshape(KO, P, NSB // 2, 2 * SBW).transpose(2, 1, 0, 3)
                ),
                "vT": np.ascontiguousarray(
                    vT.reshape(KO, P, NSB, SBW).transpose(2, 1, 0, 3)
                ),
                "wq": wq4,
                "wk": wk4,
                "wv": wv3,
                "wo": wo3,
                "bq": np.ascontiguousarray(
                    bq[d0 : d0 + DP].reshape(MT, P).T
                ).astype(np.float32),
                "bk": np.ascontiguousarray(
                    bk[d0 : d0 + DP].reshape(MT, P).T
                ).astype(np.float32),
                "bv": bv[d0 : d0 + DP].reshape(1, DP).astype(BF16),
            }
        )
    return in_maps


def assemble_yT(blocks):
    """[KO, NSB, P, SBW] blocks -> [S, D] float32."""
    yT = np.asarray(blocks, dtype=np.float32)  # [KO, NSB, P, SBW]
    return yT.transpose(0, 2, 1, 3).reshape(D, S).T


def kernel(q, k, v, Wq, bq, Wk, bk, Wv, bv, Wo, bo, use_causal_mask=1):
    from concourse.bass_utils import run_bass_kernel_spmd

    q = np.asarray(q, np.float32)
    k = np.asarray(k, np.float32)
    v = np.asarray(v, np.float32)
    Wq = np.asarray(Wq, np.float32)
    Wk = np.asarray(Wk, np.float32)
    Wv = np.asarray(Wv, np.float32)
    Wo = np.asarray(Wo, np.float32)
    bq = np.asarray(bq, np.float32)
    bk = np.asarray(bk, np.float32)
    bv = np.asarray(bv, np.float32)
    bo = np.asarray(bo, np.float32)

    nc = _get_nc()
    in_maps = make_in_maps(q, k, v, Wq, bq, Wk, bk, Wv, bv, Wo)
    trace = bool(os.environ.get("KERNEL_TRACE"))
    res = run_bass_kernel_spmd(
        nc, in_maps, core_ids=list(range(NCORES)), trace=trace
    )
    LAST_RUN.clear()
    LAST_RUN.update(
        exec_time_ns=res.exec_time_ns,
        mean_exec_time_ns=res.mean_exec_time_ns,
        trace=(res.instructions_and_trace or (None, None))[1],
        per_core_scope_times=res.per_core_scope_times,
    )

    y = np.empty((B, S, D), np.float32)
    for b in range(B):
        acc = assemble_yT(res.results[2 * b]["yT"]) + assemble_yT(
            res.results[2 * b + 1]["yT"]
        )
        y[b] = acc + bo
    return y


# revision 46
# speedup vs baseline: 1.0004x; 1.0004x over previous
"""Multi-head attention (B=4, S=2048, D=1024, H=16) on 8 TRN2 NeuronCores.

Sharding: 2D - batch 4-way x head-group 2-way. Core c handles batch b = c//2
and head group j = c%2 (8 heads, model-dim slice j*512:(j+1)*512 of the QKV
projections / rows j*512:(j+1)*512 of Wo). Each core computes a partial
[S, D] output (row-sharded Wo => partial sums); the host adds the two
partials per batch and the output bias.

Engine model (per core): ACT exp work = 8*2048^2 = 33.5M elem @ ~1.09ns
 => ~285us; PE matmul stream = attention 164us + projections 109us. The
schedule keeps ACT saturated while dribbling projection matmuls into the
per-step PE slack, in work-deadline order.

Device-side layout notes:
 - Activations kept transposed ([feature, seq]); host pre-tiles all inputs
   into [block][128, 512] contiguous chunks so every DMA is a single dense
   128KB+ transfer.
 - Attention uses the S^T layout: S^T[t, s] tiles come straight from
   lhsT=K^T, rhs=Q^T matmuls (two heads row-packed via tile_position);
   softmax-exp runs on ACT (scale=1/8 fused); V carries an appended ones
   column per head so the PV matmul also produces softmax denominators.
 - Prelude: PE warm-up matmuls + ACT table preload run during the DMA dead
   time; attention starts as soon as K(m0,s0)/Q(0,s0) land (~19us), with
   V/K/Q chains dribbled just-in-time behind the DMA arrival order.
 - Output is written as bf16 [ko][q] blocks; host reassembles, adds the two
   core partials in fp32 and the output bias.
 - The reference's "causal mask" adds log(1 + 1e-6) ~ 1e-6 to allowed
   logits - far below bf16 noise - so it is deliberately not applied.
"""

import os
import sys
import types

sys.path.insert(0, "/opt/trn_rl_repo")

import numpy as np
import ml_dtypes

B, S, D, H = 4, 2048, 1024, 16
PD = D // H          # 64 head dim
P = 128              # partitions
NCORES = 8
DP = 512             # d' (feature) slice per core = 8 heads
KO = D // P          # 8 contraction tiles for QKV projections
MT = DP // P         # 4 feature tiles (= head pairs)
NT = S // P          # 16 key/time tiles
NSB = 4              # s blocks
SBW = S // NSB       # 512 block width
HL = 8               # local heads
VW = 65              # V columns per head incl. ones column

BF16 = ml_dtypes.bfloat16

_NC = None
LAST_RUN = {}


def _install_ntff_shim():
    """bass_utils' axon trace path imports antenv.axon_hooks, which this
    image lacks; register the hook module manually so BASS_TRACE works."""
    if "antenv.axon_hooks" in sys.modules:
        return
    try:
        import trn_agent_boot.trn_boot as tb

        hook = tb._ntff_profile_via_ctypes("/opt/axon/libaxon_pjrt.so")
    except Exception:
        hook = None
    m = types.ModuleType("antenv.axon_hooks")
    m.get_axon_ntff_profile_hook = lambda: hook
    m.set_axon_ntff_profile_hook = lambda h: None
    sys.modules["antenv.axon_hooks"] = m


def _build():
    import concourse.tile as tile
    from concourse import bacc, mybir

    CDT = mybir.dt.bfloat16
    F32 = mybir.dt.float32
    EXP = mybir.ActivationFunctionType.Exp
    ADD = mybir.AluOpType.add
    MUL = mybir.AluOpType.mult

    nc = bacc.Bacc("TRN2", target_bir_lowering=False, debug=False)

    # All activations pre-tiled by the host: [ko, q, 128, 512] contiguous.
    qT_d = nc.dram_tensor("qT", [KO, NSB, P, SBW], CDT, kind="ExternalInput").ap()
    # kT pair-blocked: [ko, half, 128, 1024] -> 2KB DMA lines
    kT_d = nc.dram_tensor(
        "kT", [KO, NSB // 2, P, 2 * SBW], CDT, kind="ExternalInput"
    ).ap()
    vT_d = nc.dram_tensor("vT", [KO, NSB, P, SBW], CDT, kind="ExternalInput").ap()
    # wq/wk m-sliced: [m][128, ko, 128] contiguous per m.
    wq_d = nc.dram_tensor("wq", [MT, P, KO, P], CDT, kind="ExternalInput").ap()
    wk_d = nc.dram_tensor("wk", [MT, P, KO, P], CDT, kind="ExternalInput").ap()
    wv_d = nc.dram_tensor("wv", [P, KO, DP], CDT, kind="ExternalInput").ap()
    wo_d = nc.dram_tensor("wo", [P, MT, D], CDT, kind="ExternalInput").ap()
    bq_d = nc.dram_tensor("bq", [P, MT], F32, kind="ExternalInput").ap()
    bk_d = nc.dram_tensor("bk", [P, MT], F32, kind="ExternalInput").ap()
    bv_d = nc.dram_tensor("bv", [1, DP], CDT, kind="ExternalInput").ap()
    yT_d = nc.dram_tensor("yT", [KO, NSB, P, SBW], CDT, kind="ExternalOutput").ap()

    with tile.TileContext(nc) as tc:
        import contextlib

        with contextlib.ExitStack() as ctx:
            wkp = ctx.enter_context(tc.tile_pool(name="wk", bufs=4))
            wqp = ctx.enter_context(tc.tile_pool(name="wq", bufs=4))
            wvop = ctx.enter_context(tc.tile_pool(name="wvo", bufs=1))
            biasp = ctx.enter_context(tc.tile_pool(name="bias", bufs=1))
            kTp = ctx.enter_context(tc.tile_pool(name="kTp", bufs=16))
            vTp = ctx.enter_context(tc.tile_pool(name="vTp", bufs=12))
            qTp = ctx.enter_context(tc.tile_pool(name="qTp", bufs=16))
            inpools = {"vT": vTp, "qT": qTp}
            actp = ctx.enter_context(tc.tile_pool(name="acts", bufs=1))
            expp = ctx.enter_context(tc.tile_pool(name="exps", bufs=6))
            op = ctx.enter_context(tc.tile_pool(name="otile", bufs=6))
            bcp = ctx.enter_context(tc.tile_pool(name="bcast", bufs=1))
            yp = ctx.enter_context(tc.tile_pool(name="ystage", bufs=2))
            # PSUM budget (8 banks): S^T pair tiles 2x2 + proj/out 2 + psO 2
            ps_pair = ctx.enter_context(
                tc.tile_pool(name="ps_pair", bufs=2, space="PSUM")
            )
            ps_mm = ctx.enter_context(tc.tile_pool(name="ps_mm", bufs=2, space="PSUM"))
            ps_o = ctx.enter_context(tc.tile_pool(name="ps_o", bufs=2, space="PSUM"))

            # ---- DMA emission order is the schedule's backbone --------------
            # (~250-320 GB/s effective; every block below is a single dense
            # contiguous transfer, ordered by first use.)
            bv_sb = biasp.tile([P, DP], CDT, tag="bv")
            nc.sync.dma_start(bv_sb[:], bv_d[0:1, :].to_broadcast((P, DP)))
            bq_sb = biasp.tile([P, MT], F32, tag="bq")
            nc.sync.dma_start(bq_sb[:], bq_d[:])
            bk_sb = biasp.tile([P, MT], F32, tag="bk")
            nc.sync.dma_start(bk_sb[:], bk_d[:])

            # ---- engine warm-up (runs during the DMA dead time) -------------
            # ~10us of dummy matmuls keeps the PE HAM un-throttled (2.4GHz)
            # by the time real chains start; a dummy exp preloads the ACT
            # spline table set (~2.7us) off the critical path.
            es = expp.tile([P, 2 * SBW], CDT, tag="e", name="e_warm")
            nc.vector.memset(es[0:1, 0:32], 0.0)
            nc.scalar.activation(es[0:1, 16:32], es[0:1, 0:16], EXP)
            for i in range(48):
                ps_w = ps_mm.tile([P, SBW], F32, tag="ps", name="ps_warm")
                nc.tensor.matmul(
                    ps_w[:], bv_sb[:, 0:P], bv_sb[:, 0:DP], start=True, stop=True
                )

            wk_sb = {}
            wq_sb = {}

            def load_w(pool, w_d, dst, m, tag):
                t_sb = pool.tile([P, KO, P], CDT, tag=f"{tag}{m}")
                nc.sync.dma_start(t_sb[:], w_d[m])
                dst[m] = t_sb

            vtile = {}
            ktile = {}
            qtile = {}

            def load_block(t_dram, dst, q, tag):
                # per-ko DMAs: stripes across 8 DMA engine channels
                for ko in range(KO):
                    t_sb = inpools[tag].tile(
                        [P, SBW], CDT, tag=tag, name=f"{tag}{ko}_{q}"
                    )
                    nc.sync.dma_start(t_sb[:], t_dram[ko, q])
                    dst[(ko, q)] = t_sb

            def load_block2(t_dram, dst, q, tag):
                # kT paired (q, q+1): 2KB partition lines, 8-way striped
                for ko in range(KO):
                    t_sb = kTp.tile(
                        [P, 2 * SBW], CDT, tag=tag, name=f"{tag}{ko}_{q}"
                    )
                    nc.sync.dma_start(t_sb[:], t_dram[ko, q // 2])
                    dst[(ko, q)] = t_sb[:, 0:SBW]
                    dst[(ko, q + 1)] = t_sb[:, SBW : 2 * SBW]

            load_w(wkp, wk_d, wk_sb, 0, "wk")
            load_block2(kT_d, ktile, 0, "kT")
            load_w(wqp, wq_d, wq_sb, 0, "wq")
            load_block(qT_d, qtile, 0, "qT")
            wv_sb = wvop.tile([P, KO, DP], CDT, tag="wvo", name="wv_sb")
            nc.sync.dma_start(wv_sb[:], wv_d[:])
            load_block(vT_d, vtile, 0, "vT")
            load_block(vT_d, vtile, 1, "vT")
            load_block2(kT_d, ktile, 2, "kT")
            for m in (1, 2, 3):
                load_w(wkp, wk_d, wk_sb, m, "wk")
            for m in (1, 2, 3):
                load_w(wqp, wq_d, wq_sb, m, "wq")
            load_block(vT_d, vtile, 2, "vT")
            load_block(vT_d, vtile, 3, "vT")
            load_block(qT_d, qtile, 1, "qT")

            def q_block(ko, sbk):
                if (ko, sbk) not in qtile:
                    load_block(qT_d, qtile, sbk, "qT")
                return qtile[(ko, sbk)]

            # ---- projection building blocks --------------------------------
            Vp = actp.tile([P, NT, HL * VW], CDT, tag="Vp")
            nc.vector.memset(
                Vp[:].rearrange("p t (h c) -> p t h c", c=VW)[:, :, :, PD : PD + 1],
                1.0,
            )

            KpT = actp.tile([P, MT, S], CDT, tag="KpT")
            QpT = actp.tile([P, MT, S], CDT, tag="QpT")

            def emit_v_chain(t):
                with nc.named_scope("proj_v"):
                    ps = ps_mm.tile([P, DP], F32, tag="ps", name="ps_v")
                    for ko in range(KO):
                        nc.tensor.matmul(
                            ps[:],
                            vtile[(ko, t // 4)][:, (t % 4) * P : (t % 4 + 1) * P],
                            wv_sb[:, ko, :],
                            start=(ko == 0),
                            stop=(ko == KO - 1),
                        )
                    nc.vector.tensor_tensor(
                        Vp[:, t, :].rearrange("p (h c) -> p h c", c=VW)[:, :, 0:PD],
                        ps[:].rearrange("p (h c) -> p h c", c=PD),
                        bv_sb[:].rearrange("p (h c) -> p h c", c=PD),
                        ADD,
                    )

            def emit_k_chain(m, sbk):
                with nc.named_scope("proj_k"):
                    ps = ps_mm.tile([P, SBW], F32, tag="ps", name="ps_k")
                    for ko in range(KO):
                        nc.tensor.matmul(
                            ps[:],
                            wk_sb[m][:, ko, :],
                            ktile[(ko, sbk)][:],
                            start=(ko == 0),
                            stop=(ko == KO - 1),
                        )
                    nc.vector.tensor_scalar_add(
                        KpT[:, m, sbk * SBW : (sbk + 1) * SBW],
                        ps[:],
                        bk_sb[:, m : m + 1],
                    )

            def emit_q_chain(m, sbk):
                with nc.named_scope("proj_q"):
                    ps = ps_mm.tile([P, SBW], F32, tag="ps", name="ps_q")
                    for ko in range(KO):
                        nc.tensor.matmul(
                            ps[:],
                            wq_sb[m][:, ko, :],
                            q_block(ko, sbk)[:],
                            start=(ko == 0),
                            stop=(ko == KO - 1),
                        )
                    nc.vector.tensor_scalar_add(
                        QpT[:, m, sbk * SBW : (sbk + 1) * SBW],
                        ps[:],
                        bq_sb[:, m : m + 1],
                    )

            # ---- work-item queues (dribbled between attention steps) -------
            work_q = []   # urgent: V/K/Q projection chains, deadline-ordered
            lazy_q = []   # output-projection chains (no hard deadline)

            def push_v_chain(t):
                chain = {}

                def mk(ko, t=t, chain=chain):
                    if "ps" not in chain:
                        chain["ps"] = ps_mm.tile([P, DP], F32, tag="ps", name="ps_v")
                    ps = chain["ps"]
                    nc.tensor.matmul(
                        ps[:],
                        vtile[(ko, t // 4)][:, (t % 4) * P : (t % 4 + 1) * P],
                        wv_sb[:, ko, :],
                        start=(ko == 0),
                        stop=(ko == KO - 1),
                    )
                    if ko == KO - 1:
                        nc.vector.tensor_tensor(
                            Vp[:, t, :].rearrange("p (h c) -> p h c", c=VW)[
                                :, :, 0:PD
                            ],
                            ps[:].rearrange("p (h c) -> p h c", c=PD),
                            bv_sb[:].rearrange("p (h c) -> p h c", c=PD),
                            ADD,
                        )
                for ko in range(KO):
                    work_q.append(lambda ko=ko, mk=mk: mk(ko))

            def push_k_chain(m, sbk):
                chain = {}

                def mk(ko, m=m, sbk=sbk, chain=chain):
                    if "ps" not in chain:
                        chain["ps"] = ps_mm.tile([P, SBW], F32, tag="ps", name="ps_k")
                    ps = chain["ps"]
                    nc.tensor.matmul(
                        ps[:],
                        wk_sb[m][:, ko, :],
                        ktile[(ko, sbk)][:],
                        start=(ko == 0),
                        stop=(ko == KO - 1),
                    )
                    if ko == KO - 1:
                        nc.vector.tensor_scalar_add(
                            KpT[:, m, sbk * SBW : (sbk + 1) * SBW],
                            ps[:],
                            bk_sb[:, m : m + 1],
                        )
                for ko in range(KO):
                    work_q.append(lambda ko=ko, mk=mk: mk(ko))

            def push_q_chain(m, sbk):
                chain = {}

                def mk(ko, m=m, sbk=sbk, chain=chain):
                    if "ps" not in chain:
                        chain["ps"] = ps_mm.tile([P, SBW], F32, tag="ps", name="ps_q")
                    ps = chain["ps"]
                    nc.tensor.matmul(
                        ps[:],
                        wq_sb[m][:, ko, :],
                        q_block(ko, sbk)[:],
                        start=(ko == 0),
                        stop=(ko == KO - 1),
                    )
                    if ko == KO - 1:
                        nc.vector.tensor_scalar_add(
                            QpT[:, m, sbk * SBW : (sbk + 1) * SBW],
                            ps[:],
                            bq_sb[:, m : m + 1],
                        )
                for ko in range(KO):
                    work_q.append(lambda ko=ko, mk=mk: mk(ko))

            def push_y_chains(sb, otiles):
                for n in range(KO):
                    chain = {}

                    def mk(hp, n=n, sb=sb, otiles=otiles, chain=chain):
                        if "ps" not in chain:
                            chain["ps"] = ps_mm.tile(
                                [P, SBW], F32, tag="ps", name="ps_y"
                            )
                        psY = chain["ps"]
                        nc.tensor.matmul(
                            psY[:],
                            wo_sb[:, hp, n * P : (n + 1) * P],
                            otiles[hp][:],
                            start=(hp == 0),
                            stop=(hp == MT - 1),
                        )
                        if hp == MT - 1:
                            y_sb = yp.tile([P, SBW], CDT, tag="y")
                            nc.vector.tensor_copy(y_sb[:], psY[:])
                            nc.sync.dma_start(yT_d[n, sb], y_sb[:])
                    for hp in range(MT):
                        lazy_q.append(lambda hp=hp, mk=mk: mk(hp))

            def pull(n, lazy_ok=True):
                for _ in range(n):
                    if work_q:
                        work_q.pop(0)()
                    elif lazy_ok and lazy_q:
                        lazy_q.pop(0)()

            # ---- normalize + attention (flat (hp, t) pipeline per s-block) --
            def emit_norm(hp, psO, otiles):
                with nc.named_scope("norm"):
                    # den copies first: the broadcast+recip chain is the
                    # critical path to o_t (y-chain pulls wait on it)
                    bcd = bcp.tile([P, 2 * SBW], F32, tag="bcd")
                    nc.vector.tensor_copy(bcd[0:1, 0:SBW], psO[0][PD : PD + 1, :])
                    nc.vector.tensor_copy(
                        bcd[0:1, SBW : 2 * SBW], psO[1][PD : PD + 1, :]
                    )
                    nc.gpsimd.partition_broadcast(bcd[:], bcd[0:1, :])
                    nc.vector.reciprocal_approx_fast(bcd[:], bcd[:])
                    o_t = op.tile([P, SBW], CDT, tag="o")
                    nc.vector.tensor_tensor(
                        o_t[0:PD, :], psO[0][0:PD, :], bcd[0:PD, 0:SBW], MUL
                    )
                    nc.vector.tensor_tensor(
                        o_t[PD:P, :], psO[1][0:PD, :], bcd[PD:P, SBW : 2 * SBW], MUL
                    )
                    otiles.append(o_t)

            def emit_attn_sb(sb, otiles, rate):
                steps = [(hp, t) for hp in range(MT) for t in range(NT)]
                psO = {}
                psS = {}

                def s_mm(hp, t):
                    psT = ps_pair.tile([P, 2 * SBW], F32, tag="psT", name="psT")
                    psS[(hp, t)] = psT
                    for u in range(2):
                        nc.tensor.matmul(
                            psT[:, u * SBW : (u + 1) * SBW],
                            KpT[u * PD : (u + 1) * PD, hp, t * P : (t + 1) * P],
                            QpT[
                                u * PD : (u + 1) * PD,
                                hp,
                                sb * SBW : (sb + 1) * SBW,
                            ],
                            start=True,
                            stop=True,
                            tile_position=(u * PD, 0),
                        )

                with nc.named_scope("attn"):
                    s_mm(*steps[0])
                    for i, (hp, t) in enumerate(steps):
                        n_pull, lazy_ok = rate(i)
                        pull(n_pull, lazy_ok)
                        if t == 0:
                            psO[hp] = [
                                ps_o.tile([VW, SBW], F32, tag="psO", name=f"psO{u}")
                                for u in range(2)
                            ]
                        if i + 1 < len(steps):
                            s_mm(*steps[i + 1])
                        psT = psS.pop((hp, t))
                        e = expp.tile([P, 2 * SBW], CDT, tag="e")
                        nc.scalar.activation(
                            e[:], psT[:], EXP, scale=1.0 / np.sqrt(PD)
                        )
                        for u in range(2):
                            h = 2 * hp + u
                            nc.tensor.matmul(
                                psO[hp][u][:],
                                Vp[:, t, h * VW : (h + 1) * VW],
                                e[:, u * SBW : (u + 1) * SBW],
                                start=(t == 0),
                                stop=(t == NT - 1),
                            )
                        if t == NT - 1:
                            emit_norm(hp, psO.pop(hp), otiles)

            # ---- schedule ---------------------------------------------------
            # Inline prelude: only what gates the very first attention steps.
            emit_k_chain(0, 0)
            emit_q_chain(0, 0)
            emit_v_chain(0)
            # Everything else dribbles in DMA-arrival / deadline order.
            for t in (1, 2, 3):
                push_v_chain(t)
            push_k_chain(0, 1)
            for t in (4, 5, 6, 7):
                push_v_chain(t)
            push_k_chain(0, 2)
            for t in (8, 9, 10, 11):
                push_v_chain(t)
            push_k_chain(0, 3)
            for t in (12, 13, 14, 15):
                push_v_chain(t)
            for m in (1, 2, 3):
                push_k_chain(m, 0)
                push_q_chain(m, 0)
                for sbk in (1, 2, 3):
                    push_k_chain(m, sbk)

            # wo shares wv's pool slot; the DMA is emitted here but lands
            # late in the DMA order (v chains all emitted by then on PE).
            wo_sb = wvop.tile([P, MT, D], CDT, tag="wvo", name="wo_sb")
            nc.sync.dma_start(wo_sb[:], wo_d[:])

            def mk_rate(sb):
                def rate(i):
                    hp_step = i % NT
                    if sb == 0:
                        # deadline-matched dribble: V+K m0 in hp0, K m1-3
                        # and Q chains spread over hp1-3 (~2.5/step)
                        n = 10 if i < 16 else (3 if i % 2 == 0 else 2)
                        return (n, False)
                    # avoid stalling PE on the previous head-pair's norm
                    # chain (~5us): no lazy (y-chain) pulls until it is done.
                    return (2, hp_step >= 7)
                return rate

            otiles_by_sb = {sb: [] for sb in range(NSB)}
            for sb in range(NSB):
                if sb + 2 < NSB and (0, sb + 2) not in qtile:
                    # qT block sb+2 lands during sb, consumed during sb+1
                    load_block(qT_d, qtile, sb + 2, "qT")
                if sb + 1 < NSB:
                    for m in range(MT):
                        push_q_chain(m, sb + 1)
                emit_attn_sb(sb, otiles_by_sb[sb], mk_rate(sb))
                push_y_chains(sb, otiles_by_sb[sb])
            # keep the PE HAM warm through the final norm chain so the
            # drained output-projection matmuls run at 2.4GHz
            for i in range(8):
                ps_w = ps_pair.tile([P, SBW], F32, tag="psT", name="ps_tailwarm")
                nc.tensor.matmul(
                    ps_w[:, 0:P], bv_sb[:, 0:P], bv_sb[:, 0:P],
                    start=True, stop=True,
                )
            # drain whatever is left (final output projections)
            pull(len(work_q) + len(lazy_q))

    nc.compile()
    return nc


def _get_nc():
    global _NC
    if _NC is None:
        _install_ntff_shim()
        _NC = _build()
    return _NC


def _tile_blocks(xT):
    """[D, S] -> [KO, NSB, P, SBW] contiguous blocks."""
    return np.ascontiguousarray(
        xT.reshape(KO, P, NSB, SBW).transpose(0, 2, 1, 3)
    )


def make_in_maps(q, k, v, Wq, bq, Wk, bk, Wv, bv, Wo):
    """Shard + lay out the full inputs into the 8 per-core input maps."""
    in_maps = []
    for c in range(NCORES):
        b, j = divmod(c, 2)
        d0 = j * DP
        qT = np.ascontiguousarray(q[b].T).astype(BF16)
        kT = np.ascontiguousarray(k[b].T).astype(BF16)
        vT = np.ascontiguousarray(v[b].T).astype(BF16)
        wq = Wq[:, d0 : d0 + DP].astype(BF16)  # [D, DP]
        wk = Wk[:, d0 : d0 + DP].astype(BF16)
        wv = Wv[:, d0 : d0 + DP].astype(BF16)
        wo = Wo[d0 : d0 + DP, :].astype(BF16)  # [DP, D]
        # wq/wk m-sliced [MT, P, KO, P]: slice m covers out-cols m*128..,
        # [contraction-part p, ko, out-col] per slice.
        wq4 = np.ascontiguousarray(
            wq.reshape(KO, P, MT, P).transpose(2, 1, 0, 3)
        )
        wk4 = np.ascontiguousarray(
            wk.reshape(KO, P, MT, P).transpose(2, 1, 0, 3)
        )
        # wv [P, KO, DP]: [contraction-part, ko, out-col]
        wv3 = np.ascontiguousarray(wv.reshape(KO, P, DP).transpose(1, 0, 2))
        # wo [P, MT, D]: [contraction-part within hp, hp, out-col]
        wo3 = np.ascontiguousarray(wo.reshape(MT, P, D).transpose(1, 0, 2))
        in_maps.append(
            {
                "qT": _tile_blocks(qT),
                "kT": np.ascontiguousarray(
                    kT.reshape(KO, P, NSB // 2, 2 * SBW).transpose(0, 2, 1, 3)
                ),
                "vT": _tile_blocks(vT),
                "wq": wq4,
                "wk": wk4,
                "wv": wv3,
                "wo": wo3,
                "bq": np.ascontiguousarray(
                    bq[d0 : d0 + DP].reshape(MT, P).T
                ).astype(np.float32),
                "bk": np.ascontiguousarray(
                    bk[d0 : d0 + DP].reshape(MT, P).T
                ).astype(np.float32),
                "bv": bv[d0 : d0 + DP].reshape(1, DP).astype(BF16),
            }
        )
    return in_maps


def assemble_yT(blocks):
    """[KO, NSB, P, SBW] blocks -> [S, D] float32."""
    yT = np.asarray(blocks, dtype=np.float32)  # [KO, NSB, P, SBW]
    return yT.transpose(0, 2, 1, 3).reshape(D, S).T


def kernel(q, k, v, Wq, bq, Wk, bk, Wv, bv, Wo, bo, use_causal_mask=1):
    from concourse.bass_utils import run_bass_kernel_spmd

    q = np.asarray(q, np.float32)
    k = np.asarray(k, np.float32)
    v = np.asarray(v, np.float32)
    Wq = np.asarray(Wq, np.float32)
    Wk = np.asarray(Wk, np.float32)
    Wv = np.asarray(Wv, np.float32)
    Wo = np.asarray(Wo, np.float32)
    bq = np.asarray(bq, np.float32)
    bk = np.asarray(bk, np.float32)
    bv = np.asarray(bv, np.float32)
    bo = np.asarray(bo, np.float32)

    nc = _get_nc()
    in_maps = make_in_maps(q, k, v, Wq, bq, Wk, bk, Wv, bv, Wo)
    trace = bool(os.environ.get("KERNEL_TRACE"))
    res = run_bass_kernel_spmd(
        nc, in_maps, core_ids=list(range(NCORES)), trace=trace
    )
    LAST_RUN.clear()
    LAST_RUN.update(
        exec_time_ns=res.exec_time_ns,
        mean_exec_time_ns=res.mean_exec_time_ns,
        trace=(res.instructions_and_trace or (None, None))[1],
        per_core_scope_times=res.per_core_scope_times,
    )

    y = np.empty((B, S, D), np.float32)
    for b in range(B):
        acc = assemble_yT(res.results[2 * b]["yT"]) + assemble_yT(
            res.results[2 * b + 1]["yT"]
        )
        y[b] = acc + bo
    return y


# revision 49
# speedup vs baseline: 1.0090x; 1.0086x over previous
"""Multi-head attention (B=4, S=2048, D=1024, H=16) on 8 TRN2 NeuronCores.

Sharding: 2D - batch 4-way x head-group 2-way. Core c handles batch b = c//2
and head group j = c%2 (8 heads, model-dim slice j*512:(j+1)*512 of the QKV
projections / rows j*512:(j+1)*512 of Wo). Each core computes a partial
[S, D] output (row-sharded Wo => partial sums); the host adds the two
partials per batch and the output bias.

Engine model (per core): ACT exp work = 8*2048^2 = 33.5M elem @ ~1.09ns
 => ~285us; PE matmul stream = attention 164us + projections 109us. The
schedule keeps ACT saturated while dribbling projection matmuls into the
per-step PE slack, in work-deadline order.

Device-side layout notes:
 - Activations kept transposed ([feature, seq]); host pre-tiles all inputs
   into [block][128, 512] contiguous chunks so every DMA is a single dense
   128KB+ transfer.
 - Attention uses the S^T layout: S^T[t, s] tiles come straight from
   lhsT=K^T, rhs=Q^T matmuls (two heads row-packed via tile_position);
   softmax-exp runs on ACT (scale=1/8 fused); V carries an appended ones
   column per head so the PV matmul also produces softmax denominators.
 - Prelude: PE warm-up matmuls + ACT table preload run during the DMA dead
   time; attention starts as soon as K(m0,s0)/Q(0,s0) land (~19us), with
   V/K/Q chains dribbled just-in-time behind the DMA arrival order.
 - Output is written as bf16 [ko][q] blocks; host reassembles, adds the two
   core partials in fp32 and the output bias.
 - The reference's "causal mask" adds log(1 + 1e-6) ~ 1e-6 to allowed
   logits - far below bf16 noise - so it is deliberately not applied.
"""

import os
import sys
import types

sys.path.insert(0, "/opt/trn_rl_repo")

import numpy as np
import ml_dtypes

B, S, D, H = 4, 2048, 1024, 16
PD = D // H          # 64 head dim
P = 128              # partitions
NCORES = 8
DP = 512             # d' (feature) slice per core = 8 heads
KO = D // P          # 8 contraction tiles for QKV projections
MT = DP // P         # 4 feature tiles (= head pairs)
NT = S // P          # 16 key/time tiles
NSB = 4              # s blocks
SBW = S // NSB       # 512 block width
HL = 8               # local heads
VW = 65              # V columns per head incl. ones column

BF16 = ml_dtypes.bfloat16

_NC = None
LAST_RUN = {}


def _install_ntff_shim():
    """bass_utils' axon trace path imports antenv.axon_hooks, which this
    image lacks; register the hook module manually so BASS_TRACE works."""
    if "antenv.axon_hooks" in sys.modules:
        return
    try:
        import trn_agent_boot.trn_boot as tb

        hook = tb._ntff_profile_via_ctypes("/opt/axon/libaxon_pjrt.so")
    except Exception:
        hook = None
    m = types.ModuleType("antenv.axon_hooks")
    m.get_axon_ntff_profile_hook = lambda: hook
    m.set_axon_ntff_profile_hook = lambda h: None
    sys.modules["antenv.axon_hooks"] = m


def _build():
    import concourse.tile as tile
    from concourse import bacc, mybir

    CDT = mybir.dt.bfloat16
    F32 = mybir.dt.float32
    EXP = mybir.ActivationFunctionType.Exp
    ADD = mybir.AluOpType.add
    MUL = mybir.AluOpType.mult

    nc = bacc.Bacc("TRN2", target_bir_lowering=False, debug=False)

    # All activations pre-tiled by the host: [ko, q, 128, 512] contiguous.
    qT_d = nc.dram_tensor("qT", [KO, NSB, P, SBW], CDT, kind="ExternalInput").ap()
    # kT pair-blocked: [ko, half, 128, 1024] -> 2KB DMA lines
    kT_d = nc.dram_tensor(
        "kT", [KO, NSB // 2, P, 2 * SBW], CDT, kind="ExternalInput"
    ).ap()
    vT_d = nc.dram_tensor("vT", [KO, NSB, P, SBW], CDT, kind="ExternalInput").ap()
    # wq/wk m-sliced: [m][128, ko, 128] contiguous per m.
    wq_d = nc.dram_tensor("wq", [MT, P, KO, P], CDT, kind="ExternalInput").ap()
    wk_d = nc.dram_tensor("wk", [MT, P, KO, P], CDT, kind="ExternalInput").ap()
    wv_d = nc.dram_tensor("wv", [P, KO, DP], CDT, kind="ExternalInput").ap()
    wo_d = nc.dram_tensor("wo", [P, MT, D], CDT, kind="ExternalInput").ap()
    bq_d = nc.dram_tensor("bq", [P, MT], F32, kind="ExternalInput").ap()
    bk_d = nc.dram_tensor("bk", [P, MT], F32, kind="ExternalInput").ap()
    bv_d = nc.dram_tensor("bv", [1, DP], CDT, kind="ExternalInput").ap()
    yT_d = nc.dram_tensor("yT", [KO, NSB, P, SBW], CDT, kind="ExternalOutput").ap()

    with tile.TileContext(nc) as tc:
        import contextlib

        with contextlib.ExitStack() as ctx:
            wkp = ctx.enter_context(tc.tile_pool(name="wk", bufs=4))
            wqp = ctx.enter_context(tc.tile_pool(name="wq", bufs=4))
            wvop = ctx.enter_context(tc.tile_pool(name="wvo", bufs=1))
            biasp = ctx.enter_context(tc.tile_pool(name="bias", bufs=1))
            kTp = ctx.enter_context(tc.tile_pool(name="kTp", bufs=16))
            vTp = ctx.enter_context(tc.tile_pool(name="vTp", bufs=12))
            qTp = ctx.enter_context(tc.tile_pool(name="qTp", bufs=16))
            inpools = {"vT": vTp, "qT": qTp}
            actp = ctx.enter_context(tc.tile_pool(name="acts", bufs=1))
            warmp = ctx.enter_context(tc.tile_pool(name="warm", bufs=1))
            expp = ctx.enter_context(tc.tile_pool(name="exps", bufs=5))
            op = ctx.enter_context(tc.tile_pool(name="otile", bufs=6))
            bcp = ctx.enter_context(tc.tile_pool(name="bcast", bufs=1))
            yp = ctx.enter_context(tc.tile_pool(name="ystage", bufs=2))
            # PSUM budget (8 banks): S^T pair tiles 2x2 + proj/out 2 + psO 2
            ps_pair = ctx.enter_context(
                tc.tile_pool(name="ps_pair", bufs=2, space="PSUM")
            )
            ps_mm = ctx.enter_context(tc.tile_pool(name="ps_mm", bufs=2, space="PSUM"))
            ps_o = ctx.enter_context(tc.tile_pool(name="ps_o", bufs=2, space="PSUM"))

            # ---- DMA emission order is the schedule's backbone --------------
            # (~250-320 GB/s effective; every block below is a single dense
            # contiguous transfer, ordered by first use.)
            bv_sb = biasp.tile([P, DP], CDT, tag="bv")
            nc.sync.dma_start(bv_sb[:], bv_d[0:1, :].to_broadcast((P, DP)))
            bq_sb = biasp.tile([P, MT], F32, tag="bq")
            nc.sync.dma_start(bq_sb[:], bq_d[:])
            bk_sb = biasp.tile([P, MT], F32, tag="bk")
            nc.sync.dma_start(bk_sb[:], bk_d[:])

            # ---- engine warm-up (runs during the DMA dead time) -------------
            # ~10us of dummy matmuls keeps the PE HAM un-throttled (2.4GHz)
            # by the time real chains start; a dummy exp preloads the ACT
            # spline table set (~2.7us) off the critical path.
            wz = warmp.tile([P, SBW + 16], CDT, tag="warm")
            nc.vector.memset(wz[:], 0.0)
            nc.scalar.activation(wz[0:1, SBW : SBW + 16], wz[0:1, 0:16], EXP)
            for i in range(48):
                ps_w = ps_mm.tile([P, SBW], F32, tag="ps", name="ps_warm")
                nc.tensor.matmul(
                    ps_w[:], wz[:, 0:P], wz[:, 0:SBW], start=True, stop=True
                )

            wk_sb = {}
            wq_sb = {}

            def load_w(pool, w_d, dst, m, tag):
                t_sb = pool.tile([P, KO, P], CDT, tag=f"{tag}{m}")
                nc.sync.dma_start(t_sb[:], w_d[m])
                dst[m] = t_sb

            vtile = {}
            ktile = {}
            qtile = {}

            def load_block(t_dram, dst, q, tag):
                # per-ko DMAs: stripes across 8 DMA engine channels
                for ko in range(KO):
                    t_sb = inpools[tag].tile(
                        [P, SBW], CDT, tag=tag, name=f"{tag}{ko}_{q}"
                    )
                    nc.sync.dma_start(t_sb[:], t_dram[ko, q])
                    dst[(ko, q)] = t_sb

            def load_block2(t_dram, dst, q, tag):
                # kT paired (q, q+1): 2KB partition lines, 8-way striped
                for ko in range(KO):
                    t_sb = kTp.tile(
                        [P, 2 * SBW], CDT, tag=tag, name=f"{tag}{ko}_{q}"
                    )
                    nc.sync.dma_start(t_sb[:], t_dram[ko, q // 2])
                    dst[(ko, q)] = t_sb[:, 0:SBW]
                    dst[(ko, q + 1)] = t_sb[:, SBW : 2 * SBW]

            load_w(wkp, wk_d, wk_sb, 0, "wk")
            load_block2(kT_d, ktile, 0, "kT")
            load_w(wqp, wq_d, wq_sb, 0, "wq")
            load_block(qT_d, qtile, 0, "qT")
            wv_sb = wvop.tile([P, KO, DP], CDT, tag="wvo", name="wv_sb")
            nc.sync.dma_start(wv_sb[:], wv_d[:])
            load_block(vT_d, vtile, 0, "vT")
            load_block(vT_d, vtile, 1, "vT")
            load_block2(kT_d, ktile, 2, "kT")
            for m in (1, 2, 3):
                load_w(wkp, wk_d, wk_sb, m, "wk")
            for m in (1, 2, 3):
                load_w(wqp, wq_d, wq_sb, m, "wq")
            load_block(vT_d, vtile, 2, "vT")
            load_block(vT_d, vtile, 3, "vT")
            load_block(qT_d, qtile, 1, "qT")

            def q_block(ko, sbk):
                if (ko, sbk) not in qtile:
                    load_block(qT_d, qtile, sbk, "qT")
                return qtile[(ko, sbk)]

            # ---- projection building blocks --------------------------------
            Vp = actp.tile([P, NT, HL * VW], CDT, tag="Vp")
            nc.vector.memset(
                Vp[:].rearrange("p t (h c) -> p t h c", c=VW)[:, :, :, PD : PD + 1],
                1.0,
            )

            KpT = actp.tile([P, MT, S], CDT, tag="KpT")
            QpT = actp.tile([P, MT, S], CDT, tag="QpT")

            def emit_v_chain(t):
                with nc.named_scope("proj_v"):
                    ps = ps_mm.tile([P, DP], F32, tag="ps", name="ps_v")
                    for ko in range(KO):
                        nc.tensor.matmul(
                            ps[:],
                            vtile[(ko, t // 4)][:, (t % 4) * P : (t % 4 + 1) * P],
                            wv_sb[:, ko, :],
                            start=(ko == 0),
                            stop=(ko == KO - 1),
                        )
                    nc.vector.tensor_tensor(
                        Vp[:, t, :].rearrange("p (h c) -> p h c", c=VW)[:, :, 0:PD],
                        ps[:].rearrange("p (h c) -> p h c", c=PD),
                        bv_sb[:].rearrange("p (h c) -> p h c", c=PD),
                        ADD,
                    )

            def emit_k_chain(m, sbk):
                with nc.named_scope("proj_k"):
                    ps = ps_mm.tile([P, SBW], F32, tag="ps", name="ps_k")
                    for ko in range(KO):
                        nc.tensor.matmul(
                            ps[:],
                            wk_sb[m][:, ko, :],
                            ktile[(ko, sbk)][:],
                            start=(ko == 0),
                            stop=(ko == KO - 1),
                        )
                    nc.vector.tensor_scalar_add(
                        KpT[:, m, sbk * SBW : (sbk + 1) * SBW],
                        ps[:],
                        bk_sb[:, m : m + 1],
                    )

            def emit_q_chain(m, sbk):
                with nc.named_scope("proj_q"):
                    ps = ps_mm.tile([P, SBW], F32, tag="ps", name="ps_q")
                    for ko in range(KO):
                        nc.tensor.matmul(
                            ps[:],
                            wq_sb[m][:, ko, :],
                            q_block(ko, sbk)[:],
                            start=(ko == 0),
                            stop=(ko == KO - 1),
                        )
                    nc.vector.tensor_scalar_add(
                        QpT[:, m, sbk * SBW : (sbk + 1) * SBW],
                        ps[:],
                        bq_sb[:, m : m + 1],
                    )

            # ---- work-item queues (dribbled between attention steps) -------
            work_q = []   # urgent: V/K/Q projection chains, deadline-ordered
            lazy_q = []   # output-projection chains (no hard deadline)

            def push_v_chain(t):
                chain = {}

                def mk(ko, t=t, chain=chain):
                    if "ps" not in chain:
                        chain["ps"] = ps_mm.tile([P, DP], F32, tag="ps", name="ps_v")
                    ps = chain["ps"]
                    nc.tensor.matmul(
                        ps[:],
                        vtile[(ko, t // 4)][:, (t % 4) * P : (t % 4 + 1) * P],
                        wv_sb[:, ko, :],
                        start=(ko == 0),
                        stop=(ko == KO - 1),
                    )
                    if ko == KO - 1:
                        nc.vector.tensor_tensor(
                            Vp[:, t, :].rearrange("p (h c) -> p h c", c=VW)[
                                :, :, 0:PD
                            ],
                            ps[:].rearrange("p (h c) -> p h c", c=PD),
                            bv_sb[:].rearrange("p (h c) -> p h c", c=PD),
                            ADD,
                        )
                for ko in range(KO):
                    work_q.append(lambda ko=ko, mk=mk: mk(ko))

            def push_k_chain(m, sbk):
                chain = {}

                def mk(ko, m=m, sbk=sbk, chain=chain):
                    if "ps" not in chain:
                        chain["ps"] = ps_mm.tile([P, SBW], F32, tag="ps", name="ps_k")
                    ps = chain["ps"]
                    nc.tensor.matmul(
                        ps[:],
                        wk_sb[m][:, ko, :],
                        ktile[(ko, sbk)][:],
                        start=(ko == 0),
                        stop=(ko == KO - 1),
                    )
                    if ko == KO - 1:
                        nc.vector.tensor_scalar_add(
                            KpT[:, m, sbk * SBW : (sbk + 1) * SBW],
                            ps[:],
                            bk_sb[:, m : m + 1],
                        )
                for ko in range(KO):
                    work_q.append(lambda ko=ko, mk=mk: mk(ko))

            def push_q_chain(m, sbk):
                chain = {}

                def mk(ko, m=m, sbk=sbk, chain=chain):
                    if "ps" not in chain:
                        chain["ps"] = ps_mm.tile([P, SBW], F32, tag="ps", name="ps_q")
                    ps = chain["ps"]
                    nc.tensor.matmul(
                        ps[:],
                        wq_sb[m][:, ko, :],
                        q_block(ko, sbk)[:],
                        start=(ko == 0),
                        stop=(ko == KO - 1),
                    )
                    if ko == KO - 1:
                        nc.vector.tensor_scalar_add(
                            QpT[:, m, sbk * SBW : (sbk + 1) * SBW],
                            ps[:],
                            bq_sb[:, m : m + 1],
                        )
                for ko in range(KO):
                    work_q.append(lambda ko=ko, mk=mk: mk(ko))

            def push_y_chains(sb, otiles):
                for n in range(KO):
                    chain = {}

                    def mk(hp, n=n, sb=sb, otiles=otiles, chain=chain):
                        if "ps" not in chain:
                            chain["ps"] = ps_mm.tile(
                                [P, SBW], F32, tag="ps", name="ps_y"
                            )
                        psY = chain["ps"]
                        nc.tensor.matmul(
                            psY[:],
                            wo_sb[:, hp, n * P : (n + 1) * P],
                            otiles[hp][:],
                            start=(hp == 0),
                            stop=(hp == MT - 1),
                        )
                        if hp == MT - 1:
                            y_sb = yp.tile([P, SBW], CDT, tag="y")
                            nc.vector.tensor_copy(y_sb[:], psY[:])
                            nc.sync.dma_start(yT_d[n, sb], y_sb[:])
                    for hp in range(MT):
                        lazy_q.append(lambda hp=hp, mk=mk: mk(hp))

            def pull(n, lazy_ok=True):
                for _ in range(n):
                    if work_q:
                        work_q.pop(0)()
                    elif lazy_ok and lazy_q:
                        lazy_q.pop(0)()

            # ---- normalize + attention (flat (hp, t) pipeline per s-block) --
            def emit_norm(hp, psO, otiles):
                with nc.named_scope("norm"):
                    # den copies first: the broadcast+recip chain is the
                    # critical path to o_t (y-chain pulls wait on it)
                    bcd = bcp.tile([P, 2 * SBW], F32, tag="bcd")
                    nc.vector.tensor_copy(bcd[0:1, 0:SBW], psO[0][PD : PD + 1, :])
                    nc.vector.tensor_copy(
                        bcd[0:1, SBW : 2 * SBW], psO[1][PD : PD + 1, :]
                    )
                    nc.gpsimd.partition_broadcast(bcd[:], bcd[0:1, :])
                    nc.vector.reciprocal_approx_fast(bcd[:], bcd[:])
                    o_t = op.tile([P, SBW], CDT, tag="o")
                    nc.vector.tensor_tensor(
                        o_t[0:PD, :], psO[0][0:PD, :], bcd[0:PD, 0:SBW], MUL
                    )
                    nc.vector.tensor_tensor(
                        o_t[PD:P, :], psO[1][0:PD, :], bcd[PD:P, SBW : 2 * SBW], MUL
                    )
                    otiles.append(o_t)

            def emit_attn_sb(sb, otiles, rate):
                steps = [(hp, t) for hp in range(MT) for t in range(NT)]
                psO = {}
                psS = {}

                def s_mm(hp, t):
                    psT = ps_pair.tile([P, 2 * SBW], F32, tag="psT", name="psT")
                    psS[(hp, t)] = psT
                    for u in range(2):
                        nc.tensor.matmul(
                            psT[:, u * SBW : (u + 1) * SBW],
                            KpT[u * PD : (u + 1) * PD, hp, t * P : (t + 1) * P],
                            QpT[
                                u * PD : (u + 1) * PD,
                                hp,
                                sb * SBW : (sb + 1) * SBW,
                            ],
                            start=True,
                            stop=True,
                            tile_position=(u * PD, 0),
                        )

                with nc.named_scope("attn"):
                    s_mm(*steps[0])
                    for i, (hp, t) in enumerate(steps):
                        n_pull, lazy_ok = rate(i)
                        pull(n_pull, lazy_ok)
                        if t == 0:
                            psO[hp] = [
                                ps_o.tile([VW, SBW], F32, tag="psO", name=f"psO{u}")
                                for u in range(2)
                            ]
                        if i + 1 < len(steps):
                            s_mm(*steps[i + 1])
                        psT = psS.pop((hp, t))
                        e = expp.tile([P, 2 * SBW], CDT, tag="e")
                        nc.scalar.activation(
                            e[:], psT[:], EXP, scale=1.0 / np.sqrt(PD)
                        )
                        for u in range(2):
                            h = 2 * hp + u
                            nc.tensor.matmul(
                                psO[hp][u][:],
                                Vp[:, t, h * VW : (h + 1) * VW],
                                e[:, u * SBW : (u + 1) * SBW],
                                start=(t == 0),
                                stop=(t == NT - 1),
                            )
                        if t == NT - 1:
                            emit_norm(hp, psO.pop(hp), otiles)

            # ---- schedule ---------------------------------------------------
            # Inline prelude: only what gates the very first attention steps.
            emit_k_chain(0, 0)
            emit_q_chain(0, 0)
            emit_v_chain(0)
            # Everything else dribbles in DMA-arrival / deadline order.
            for t in (1, 2, 3):
                push_v_chain(t)
            push_k_chain(0, 1)
            for t in (4, 5, 6, 7):
                push_v_chain(t)
            push_k_chain(0, 2)
            for t in (8, 9, 10, 11):
                push_v_chain(t)
            push_k_chain(0, 3)
            for t in (12, 13, 14, 15):
                push_v_chain(t)
            for m in (1, 2, 3):
                push_k_chain(m, 0)
                push_q_chain(m, 0)
                for sbk in (1, 2, 3):
                    push_k_chain(m, sbk)

            # wo shares wv's pool slot; the DMA is emitted here but lands
            # late in the DMA order (v chains all emitted by then on PE).
            wo_sb = wvop.tile([P, MT, D], CDT, tag="wvo", name="wo_sb")
            nc.sync.dma_start(wo_sb[:], wo_d[:])

            def mk_rate(sb):
                def rate(i):
                    hp_step = i % NT
                    if sb == 0:
                        # deadline-matched dribble: V+K m0 in hp0, K m1-3
                        # and Q chains spread over hp1-3 (~2.5/step)
                        n = 10 if i < 16 else (3 if i % 2 == 0 else 2)
                        return (n, False)
                    # avoid stalling PE on the previous head-pair's norm
                    # chain (~5us): no lazy (y-chain) pulls until it is done.
                    return (2, hp_step >= 7)
                return rate

            otiles_by_sb = {sb: [] for sb in range(NSB)}
            for sb in range(NSB):
                if sb + 2 < NSB and (0, sb + 2) not in qtile:
                    # qT block sb+2 lands during sb, consumed during sb+1
                    load_block(qT_d, qtile, sb + 2, "qT")
                if sb + 1 < NSB:
                    for m in range(MT):
                        push_q_chain(m, sb + 1)
                emit_attn_sb(sb, otiles_by_sb[sb], mk_rate(sb))
                push_y_chains(sb, otiles_by_sb[sb])
            # keep the PE HAM warm through the final norm chain so the
            # drained output-projection matmuls run at 2.4GHz
            for i in range(8):
                ps_w = ps_pair.tile([P, SBW], F32, tag="psT", name="ps_tailwarm")
                nc.tensor.matmul(
                    ps_w[:, 0:P], wz[:, 0:P], wz[:, 0:P], start=True, stop=True
                )
            # drain whatever is left (final output projections)
            pull(len(work_q) + len(lazy_q))

    nc.compile()
    return nc


def _get_nc():
    global _NC
    if _NC is None:
        _install_ntff_shim()
        _NC = _build()
    return _NC


def _tile_blocks(xT):
    """[D, S] -> [KO, NSB, P, SBW] contiguous blocks."""
    return np.ascontiguousarray(
        xT.reshape(KO, P, NSB, SBW).transpose(0, 2, 1, 3)
    )


def make_in_maps(q, k, v, Wq, bq, Wk, bk, Wv, bv, Wo):
    """Shard + lay out the full inputs into the 8 per-core input maps."""
    in_maps = []
    for c in range(NCORES):
        b, j = divmod(c, 2)
        d0 = j * DP
        qT = np.ascontiguousarray(q[b].T).astype(BF16)
        kT = np.ascontiguousarray(k[b].T).astype(BF16)
        vT = np.ascontiguousarray(v[b].T).astype(BF16)
        wq = Wq[:, d0 : d0 + DP].astype(BF16)  # [D, DP]
        wk = Wk[:, d0 : d0 + DP].astype(BF16)
        wv = Wv[:, d0 : d0 + DP].astype(BF16)
        wo = Wo[d0 : d0 + DP, :].astype(BF16)  # [DP, D]
        # wq/wk m-sliced [MT, P, KO, P]: slice m covers out-cols m*128..,
        # [contraction-part p, ko, out-col] per slice.
        wq4 = np.ascontiguousarray(
            wq.reshape(KO, P, MT, P).transpose(2, 1, 0, 3)
        )
        wk4 = np.ascontiguousarray(
            wk.reshape(KO, P, MT, P).transpose(2, 1, 0, 3)
        )
        # wv [P, KO, DP]: [contraction-part, ko, out-col]
        wv3 = np.ascontiguousarray(wv.reshape(KO, P, DP).transpose(1, 0, 2))
        # wo [P, MT, D]: [contraction-part within hp, hp, out-col]
        wo3 = np.ascontiguousarray(wo.reshape(MT, P, D).transpose(1, 0, 2))
        in_maps.append(
            {
                "qT": _tile_blocks(qT),
                "kT": np.ascontiguousarray(
                    kT.reshape(KO, P, NSB // 2, 2 * SBW).transpose(0, 2, 1, 3)
                ),
                "vT": _tile_blocks(vT),
                "wq": wq4,
                "wk": wk4,
                "wv": wv3,
                "wo": wo3,
                "bq": np.ascontiguousarray(
                    bq[d0 : d0 + DP].reshape(MT, P).T
                ).astype(np.float32),
                "bk": np.ascontiguousarray(
                    bk[d0 : d0 + DP].reshape(MT, P).T
                ).astype(np.float32),
                "bv": bv[d0 : d0 + DP].reshape(1, DP).astype(BF16),
            }
        )
    return in_maps


def assemble_yT(blocks):
    """[KO, NSB, P, SBW] blocks -> [S, D] float32."""
    yT = np.asarray(blocks, dtype=np.float32)  # [KO, NSB, P, SBW]
    return yT.transpose(0, 2, 1, 3).reshape(D, S).T


def kernel(q, k, v, Wq, bq, Wk, bk, Wv, bv, Wo, bo, use_causal_mask=1):
    from concourse.bass_utils import run_bass_kernel_spmd

    q = np.asarray(q, np.float32)
    k = np.asarray(k, np.float32)
    v = np.asarray(v, np.float32)
    Wq = np.asarray(Wq, np.float32)
    Wk = np.asarray(Wk, np.float32)
    Wv = np.asarray(Wv, np.float32)
    Wo = np.asarray(Wo, np.float32)
    bq = np.asarray(bq, np.float32)
    bk = np.asarray(bk, np.float32)
    bv = np.asarray(bv, np.float32)
    bo = np.asarray(bo, np.float32)

    nc = _get_nc()
    in_maps = make_in_maps(q, k, v, Wq, bq, Wk, bk, Wv, bv, Wo)
    trace = bool(os.environ.get("KERNEL_TRACE"))
    res = run_bass_kernel_spmd(
        nc, in_maps, core_ids=list(range(NCORES)), trace=trace
    )
    LAST_RUN.clear()
    LAST_RUN.update(
        exec_time_ns=res.exec_time_ns,
        mean_exec_time_ns=res.mean_exec_time_ns,
        trace=(res.instructions_and_trace or (None, None))[1],
        per_core_scope_times=res.per_core_scope_times,
    )

    y = np.empty((B, S, D), np.float32)
    for b in range(B):
        acc = assemble_yT(res.results[2 * b]["yT"]) + assemble_yT(
            res.results[2 * b + 1]["yT"]
        )
        y[b] = acc + bo
    return y


# revision 51
# speedup vs baseline: 1.0760x; 1.0663x over previous
"""Multi-head attention (B=4, S=2048, D=1024, H=16) on 8 TRN2 NeuronCores.

Sharding: 2D - batch 4-way x head-group 2-way. Core c handles batch b = c//2
and head group j = c%2 (8 heads, model-dim slice j*512:(j+1)*512 of the QKV
projections / rows j*512:(j+1)*512 of Wo). Each core computes a partial
[S, D] output (row-sharded Wo => partial sums); the host adds the two
partials per batch and the output bias.

Engine model (per core): ACT exp work = 8*2048^2 = 33.5M elem @ ~1.09ns
 => ~285us; PE matmul stream = attention 164us + projections 109us. The
schedule keeps ACT saturated while dribbling projection matmuls into the
per-step PE slack, in work-deadline order.

Device-side layout notes:
 - Activations kept transposed ([feature, seq]); host pre-tiles all inputs
   into [block][128, 512] contiguous chunks so every DMA is a single dense
   128KB+ transfer.
 - Attention uses the S^T layout: S^T[t, s] tiles come straight from
   lhsT=K^T, rhs=Q^T matmuls (two heads row-packed via tile_position);
   softmax-exp runs on ACT (scale=1/8 fused); V carries an appended ones
   column per head so the PV matmul also produces softmax denominators.
 - Prelude: PE warm-up matmuls + ACT table preload run during the DMA dead
   time; attention starts as soon as K(m0,s0)/Q(0,s0) land (~19us), with
   V/K/Q chains dribbled just-in-time behind the DMA arrival order.
 - Output is written as bf16 [ko][q] blocks; host reassembles, adds the two
   core partials in fp32 and the output bias.
 - The reference's "causal mask" adds log(1 + 1e-6) ~ 1e-6 to allowed
   logits - far below bf16 noise - so it is deliberately not applied.
"""

import os
import sys
import types

sys.path.insert(0, "/opt/trn_rl_repo")

import numpy as np
import ml_dtypes

B, S, D, H = 4, 2048, 1024, 16
PD = D // H          # 64 head dim
P = 128              # partitions
NCORES = 8
DP = 512             # d' (feature) slice per core = 8 heads
KO = D // P          # 8 contraction tiles for QKV projections
MT = DP // P         # 4 feature tiles (= head pairs)
NT = S // P          # 16 key/time tiles
NSB = 4              # s blocks
SBW = S // NSB       # 512 block width
HL = 8               # local heads
VW = 65              # V columns per head incl. ones column

BF16 = ml_dtypes.bfloat16

_NC = None
LAST_RUN = {}


def _install_ntff_shim():
    """bass_utils' axon trace path imports antenv.axon_hooks, which this
    image lacks; register the hook module manually so BASS_TRACE works."""
    if "antenv.axon_hooks" in sys.modules:
        return
    try:
        import trn_agent_boot.trn_boot as tb

        hook = tb._ntff_profile_via_ctypes("/opt/axon/libaxon_pjrt.so")
    except Exception:
        hook = None
    m = types.ModuleType("antenv.axon_hooks")
    m.get_axon_ntff_profile_hook = lambda: hook
    m.set_axon_ntff_profile_hook = lambda h: None
    sys.modules["antenv.axon_hooks"] = m


def _build():
    import concourse.tile as tile
    from concourse import bacc, mybir

    CDT = mybir.dt.bfloat16
    F32 = mybir.dt.float32
    EXP = mybir.ActivationFunctionType.Exp
    ADD = mybir.AluOpType.add
    MUL = mybir.AluOpType.mult

    nc = bacc.Bacc("TRN2", target_bir_lowering=False, debug=False)

    # All activations pre-tiled by the host: [ko, q, 128, 512] contiguous.
    qT_d = nc.dram_tensor("qT", [KO, NSB, P, SBW], CDT, kind="ExternalInput").ap()
    # kT pair-blocked: [ko, half, 128, 1024] -> 2KB DMA lines
    kT_d = nc.dram_tensor(
        "kT", [KO, NSB // 2, P, 2 * SBW], CDT, kind="ExternalInput"
    ).ap()
    vT_d = nc.dram_tensor("vT", [KO, NSB, P, SBW], CDT, kind="ExternalInput").ap()
    # wq/wk m-sliced: [m][128, ko, 128] contiguous per m.
    wq_d = nc.dram_tensor("wq", [MT, P, KO, P], CDT, kind="ExternalInput").ap()
    wk_d = nc.dram_tensor("wk", [MT, P, KO, P], CDT, kind="ExternalInput").ap()
    wv_d = nc.dram_tensor("wv", [P, KO, DP], CDT, kind="ExternalInput").ap()
    wo_d = nc.dram_tensor("wo", [P, MT, D], CDT, kind="ExternalInput").ap()
    bq_d = nc.dram_tensor("bq", [P, MT], F32, kind="ExternalInput").ap()
    bk_d = nc.dram_tensor("bk", [P, MT], F32, kind="ExternalInput").ap()
    bv_d = nc.dram_tensor("bv", [1, DP], CDT, kind="ExternalInput").ap()
    yT_d = nc.dram_tensor("yT", [KO, NSB, P, SBW], CDT, kind="ExternalOutput").ap()

    with tile.TileContext(nc) as tc:
        import contextlib

        with contextlib.ExitStack() as ctx:
            wkp = ctx.enter_context(tc.tile_pool(name="wk", bufs=4))
            wqp = ctx.enter_context(tc.tile_pool(name="wq", bufs=4))
            wvop = ctx.enter_context(tc.tile_pool(name="wvo", bufs=1))
            biasp = ctx.enter_context(tc.tile_pool(name="bias", bufs=1))
            kTp = ctx.enter_context(tc.tile_pool(name="kTp", bufs=16))
            vTp = ctx.enter_context(tc.tile_pool(name="vTp", bufs=12))
            qTp = ctx.enter_context(tc.tile_pool(name="qTp", bufs=16))
            inpools = {"vT": vTp, "qT": qTp}
            actp = ctx.enter_context(tc.tile_pool(name="acts", bufs=1))
            warmp = ctx.enter_context(tc.tile_pool(name="warm", bufs=1))
            expp = ctx.enter_context(tc.tile_pool(name="exps", bufs=5))
            op = ctx.enter_context(tc.tile_pool(name="otile", bufs=6))
            orp = ctx.enter_context(tc.tile_pool(name="oraw", bufs=1))
            bcp = ctx.enter_context(tc.tile_pool(name="bcast", bufs=1))
            yp = ctx.enter_context(tc.tile_pool(name="ystage", bufs=2))
            # PSUM budget (8 banks): S^T pair tiles 2x2 + proj/out 2 + psO 2
            ps_pair = ctx.enter_context(
                tc.tile_pool(name="ps_pair", bufs=2, space="PSUM")
            )
            ps_mm = ctx.enter_context(tc.tile_pool(name="ps_mm", bufs=2, space="PSUM"))
            ps_o = ctx.enter_context(tc.tile_pool(name="ps_o", bufs=2, space="PSUM"))

            # ---- DMA emission order is the schedule's backbone --------------
            # (~250-320 GB/s effective; every block below is a single dense
            # contiguous transfer, ordered by first use.)
            bv_sb = biasp.tile([P, DP], CDT, tag="bv")
            nc.sync.dma_start(bv_sb[:], bv_d[0:1, :].to_broadcast((P, DP)))
            bq_sb = biasp.tile([P, MT], F32, tag="bq")
            nc.sync.dma_start(bq_sb[:], bq_d[:])
            bk_sb = biasp.tile([P, MT], F32, tag="bk")
            nc.sync.dma_start(bk_sb[:], bk_d[:])

            # ---- engine warm-up (runs during the DMA dead time) -------------
            # ~10us of dummy matmuls keeps the PE HAM un-throttled (2.4GHz)
            # by the time real chains start; a dummy exp preloads the ACT
            # spline table set (~2.7us) off the critical path.
            wz = warmp.tile([P, SBW + 16], CDT, tag="warm")
            nc.vector.memset(wz[:], 0.0)
            nc.scalar.activation(wz[0:1, SBW : SBW + 16], wz[0:1, 0:16], EXP)
            for i in range(48):
                ps_w = ps_mm.tile([P, SBW], F32, tag="ps", name="ps_warm")
                nc.tensor.matmul(
                    ps_w[:], wz[:, 0:P], wz[:, 0:SBW], start=True, stop=True
                )

            wk_sb = {}
            wq_sb = {}

            def load_w(pool, w_d, dst, m, tag):
                t_sb = pool.tile([P, KO, P], CDT, tag=f"{tag}{m}")
                nc.sync.dma_start(t_sb[:], w_d[m])
                dst[m] = t_sb

            vtile = {}
            ktile = {}
            qtile = {}

            def load_block(t_dram, dst, q, tag):
                # per-ko DMAs: stripes across 8 DMA engine channels
                for ko in range(KO):
                    t_sb = inpools[tag].tile(
                        [P, SBW], CDT, tag=tag, name=f"{tag}{ko}_{q}"
                    )
                    nc.sync.dma_start(t_sb[:], t_dram[ko, q])
                    dst[(ko, q)] = t_sb

            def load_block2(t_dram, dst, q, tag):
                # kT paired (q, q+1): 2KB partition lines, 8-way striped
                for ko in range(KO):
                    t_sb = kTp.tile(
                        [P, 2 * SBW], CDT, tag=tag, name=f"{tag}{ko}_{q}"
                    )
                    nc.sync.dma_start(t_sb[:], t_dram[ko, q // 2])
                    dst[(ko, q)] = t_sb[:, 0:SBW]
                    dst[(ko, q + 1)] = t_sb[:, SBW : 2 * SBW]

            load_w(wkp, wk_d, wk_sb, 0, "wk")
            load_block2(kT_d, ktile, 0, "kT")
            load_w(wqp, wq_d, wq_sb, 0, "wq")
            load_block(qT_d, qtile, 0, "qT")
            wv_sb = wvop.tile([P, KO, DP], CDT, tag="wvo", name="wv_sb")
            nc.sync.dma_start(wv_sb[:], wv_d[:])
            load_block(vT_d, vtile, 0, "vT")
            load_block(vT_d, vtile, 1, "vT")
            load_block2(kT_d, ktile, 2, "kT")
            for m in (1, 2, 3):
                load_w(wkp, wk_d, wk_sb, m, "wk")
            for m in (1, 2, 3):
                load_w(wqp, wq_d, wq_sb, m, "wq")
            load_block(vT_d, vtile, 2, "vT")
            load_block(vT_d, vtile, 3, "vT")
            load_block(qT_d, qtile, 1, "qT")

            def q_block(ko, sbk):
                if (ko, sbk) not in qtile:
                    load_block(qT_d, qtile, sbk, "qT")
                return qtile[(ko, sbk)]

            # ---- projection building blocks --------------------------------
            Vp = actp.tile([P, NT, HL * VW], CDT, tag="Vp")
            nc.vector.memset(
                Vp[:].rearrange("p t (h c) -> p t h c", c=VW)[:, :, :, PD : PD + 1],
                1.0,
            )

            KpT = actp.tile([P, MT, S], CDT, tag="KpT")
            QpT = actp.tile([P, MT, S], CDT, tag="QpT")

            def emit_v_chain(t):
                with nc.named_scope("proj_v"):
                    ps = ps_mm.tile([P, DP], F32, tag="ps", name="ps_v")
                    for ko in range(KO):
                        nc.tensor.matmul(
                            ps[:],
                            vtile[(ko, t // 4)][:, (t % 4) * P : (t % 4 + 1) * P],
                            wv_sb[:, ko, :],
                            start=(ko == 0),
                            stop=(ko == KO - 1),
                        )
                    nc.vector.tensor_tensor(
                        Vp[:, t, :].rearrange("p (h c) -> p h c", c=VW)[:, :, 0:PD],
                        ps[:].rearrange("p (h c) -> p h c", c=PD),
                        bv_sb[:].rearrange("p (h c) -> p h c", c=PD),
                        ADD,
                    )

            def emit_k_chain(m, sbk):
                with nc.named_scope("proj_k"):
                    ps = ps_mm.tile([P, SBW], F32, tag="ps", name="ps_k")
                    for ko in range(KO):
                        nc.tensor.matmul(
                            ps[:],
                            wk_sb[m][:, ko, :],
                            ktile[(ko, sbk)][:],
                            start=(ko == 0),
                            stop=(ko == KO - 1),
                        )
                    nc.vector.tensor_scalar_add(
                        KpT[:, m, sbk * SBW : (sbk + 1) * SBW],
                        ps[:],
                        bk_sb[:, m : m + 1],
                    )

            def emit_q_chain(m, sbk):
                with nc.named_scope("proj_q"):
                    ps = ps_mm.tile([P, SBW], F32, tag="ps", name="ps_q")
                    for ko in range(KO):
                        nc.tensor.matmul(
                            ps[:],
                            wq_sb[m][:, ko, :],
                            q_block(ko, sbk)[:],
                            start=(ko == 0),
                            stop=(ko == KO - 1),
                        )
                    nc.vector.tensor_scalar_add(
                        QpT[:, m, sbk * SBW : (sbk + 1) * SBW],
                        ps[:],
                        bq_sb[:, m : m + 1],
                    )

            # ---- work-item queues (dribbled between attention steps) -------
            work_q = []   # urgent: V/K/Q projection chains, deadline-ordered
            lazy_q = []   # output-projection chains (no hard deadline)

            def push_v_chain(t):
                chain = {}

                def mk(ko, t=t, chain=chain):
                    if "ps" not in chain:
                        chain["ps"] = ps_mm.tile([P, DP], F32, tag="ps", name="ps_v")
                    ps = chain["ps"]
                    nc.tensor.matmul(
                        ps[:],
                        vtile[(ko, t // 4)][:, (t % 4) * P : (t % 4 + 1) * P],
                        wv_sb[:, ko, :],
                        start=(ko == 0),
                        stop=(ko == KO - 1),
                    )
                    if ko == KO - 1:
                        nc.vector.tensor_tensor(
                            Vp[:, t, :].rearrange("p (h c) -> p h c", c=VW)[
                                :, :, 0:PD
                            ],
                            ps[:].rearrange("p (h c) -> p h c", c=PD),
                            bv_sb[:].rearrange("p (h c) -> p h c", c=PD),
                            ADD,
                        )
                for ko in range(KO):
                    work_q.append(lambda ko=ko, mk=mk: mk(ko))

            def push_k_chain(m, sbk):
                chain = {}

                def mk(ko, m=m, sbk=sbk, chain=chain):
                    if "ps" not in chain:
                        chain["ps"] = ps_mm.tile([P, SBW], F32, tag="ps", name="ps_k")
                    ps = chain["ps"]
                    nc.tensor.matmul(
                        ps[:],
                        wk_sb[m][:, ko, :],
                        ktile[(ko, sbk)][:],
                        start=(ko == 0),
                        stop=(ko == KO - 1),
                    )
                    if ko == KO - 1:
                        nc.vector.tensor_scalar_add(
                            KpT[:, m, sbk * SBW : (sbk + 1) * SBW],
                            ps[:],
                            bk_sb[:, m : m + 1],
                        )
                for ko in range(KO):
                    work_q.append(lambda ko=ko, mk=mk: mk(ko))

            def push_q_chain(m, sbk):
                chain = {}

                def mk(ko, m=m, sbk=sbk, chain=chain):
                    if "ps" not in chain:
                        chain["ps"] = ps_mm.tile([P, SBW], F32, tag="ps", name="ps_q")
                    ps = chain["ps"]
                    nc.tensor.matmul(
                        ps[:],
                        wq_sb[m][:, ko, :],
                        q_block(ko, sbk)[:],
                        start=(ko == 0),
                        stop=(ko == KO - 1),
                    )
                    if ko == KO - 1:
                        nc.vector.tensor_scalar_add(
                            QpT[:, m, sbk * SBW : (sbk + 1) * SBW],
                            ps[:],
                            bq_sb[:, m : m + 1],
                        )
                for ko in range(KO):
                    work_q.append(lambda ko=ko, mk=mk: mk(ko))

            def push_y_chains(sb, otiles):
                for n in range(KO):
                    chain = {}

                    def mk(hp, n=n, sb=sb, otiles=otiles, chain=chain):
                        if "ps" not in chain:
                            chain["ps"] = ps_mm.tile(
                                [P, SBW], F32, tag="ps", name="ps_y"
                            )
                        psY = chain["ps"]
                        nc.tensor.matmul(
                            psY[:],
                            wo_sb[:, hp, n * P : (n + 1) * P],
                            otiles[hp][:],
                            start=(hp == 0),
                            stop=(hp == MT - 1),
                        )
                        if hp == MT - 1:
                            y_sb = yp.tile([P, SBW], CDT, tag="y")
                            nc.vector.tensor_copy(y_sb[:], psY[:])
                            nc.sync.dma_start(yT_d[n, sb], y_sb[:])
                    for hp in range(MT):
                        lazy_q.append(lambda hp=hp, mk=mk: mk(hp))

            def pull(n, lazy_ok=True):
                for _ in range(n):
                    if work_q:
                        work_q.pop(0)()
                    elif lazy_ok and lazy_q:
                        lazy_q.pop(0)()

            # ---- normalize + attention (flat (hp, t) pipeline per s-block) --
            def emit_norm(hp, psO, otiles):
                with nc.named_scope("norm"):
                    # den copies first: the broadcast+recip chain is the
                    # critical path to o_t (y-chain pulls wait on it)
                    bcd = bcp.tile([P, 2 * SBW], F32, tag="bcd")
                    nc.vector.tensor_copy(bcd[0:1, 0:SBW], psO[0][PD : PD + 1, :])
                    nc.vector.tensor_copy(
                        bcd[0:1, SBW : 2 * SBW], psO[1][PD : PD + 1, :]
                    )
                    nc.gpsimd.partition_broadcast(bcd[:], bcd[0:1, :])
                    oraw = orp.tile([P, SBW], F32, tag="oraw")
                    nc.vector.tensor_copy(oraw[0:PD, :], psO[0][0:PD, :])
                    nc.vector.tensor_copy(oraw[PD:P, :], psO[1][0:PD, :])
                    nc.vector.reciprocal_approx_fast(bcd[:], bcd[:])
                    o_t = op.tile([P, SBW], CDT, tag="o")
                    nc.vector.tensor_tensor(
                        o_t[0:PD, :], oraw[0:PD, :], bcd[0:PD, 0:SBW], MUL
                    )
                    nc.vector.tensor_tensor(
                        o_t[PD:P, :], oraw[PD:P, :], bcd[PD:P, SBW : 2 * SBW], MUL
                    )
                    otiles.append(o_t)

            def emit_attn_sb(sb, otiles, rate):
                steps = [(hp, t) for hp in range(MT) for t in range(NT)]
                psO = {}
                psS = {}

                def s_mm(hp, t):
                    psT = ps_pair.tile([P, 2 * SBW], F32, tag="psT", name="psT")
                    psS[(hp, t)] = psT
                    for u in range(2):
                        nc.tensor.matmul(
                            psT[:, u * SBW : (u + 1) * SBW],
                            KpT[u * PD : (u + 1) * PD, hp, t * P : (t + 1) * P],
                            QpT[
                                u * PD : (u + 1) * PD,
                                hp,
                                sb * SBW : (sb + 1) * SBW,
                            ],
                            start=True,
                            stop=True,
                            tile_position=(u * PD, 0),
                        )

                with nc.named_scope("attn"):
                    s_mm(*steps[0])
                    for i, (hp, t) in enumerate(steps):
                        n_pull, lazy_ok = rate(i)
                        pull(n_pull, lazy_ok)
                        if t == 0:
                            psO[hp] = [
                                ps_o.tile([VW, SBW], F32, tag="psO", name=f"psO{u}")
                                for u in range(2)
                            ]
                        if i + 1 < len(steps):
                            s_mm(*steps[i + 1])
                        psT = psS.pop((hp, t))
                        e = expp.tile([P, 2 * SBW], CDT, tag="e")
                        nc.scalar.activation(
                            e[:], psT[:], EXP, scale=1.0 / np.sqrt(PD)
                        )
                        for u in range(2):
                            h = 2 * hp + u
                            nc.tensor.matmul(
                                psO[hp][u][:],
                                Vp[:, t, h * VW : (h + 1) * VW],
                                e[:, u * SBW : (u + 1) * SBW],
                                start=(t == 0),
                                stop=(t == NT - 1),
                            )
                        if t == NT - 1:
                            emit_norm(hp, psO.pop(hp), otiles)

            # ---- schedule ---------------------------------------------------
            # Inline prelude: only what gates the very first attention steps.
            emit_k_chain(0, 0)
            emit_q_chain(0, 0)
            emit_v_chain(0)
            # Everything else dribbles in DMA-arrival / deadline order.
            for t in (1, 2, 3):
                push_v_chain(t)
            push_k_chain(0, 1)
            for t in (4, 5, 6, 7):
                push_v_chain(t)
            push_k_chain(0, 2)
            for t in (8, 9, 10, 11):
                push_v_chain(t)
            push_k_chain(0, 3)
            for t in (12, 13, 14, 15):
                push_v_chain(t)
            for m in (1, 2, 3):
                push_k_chain(m, 0)
                push_q_chain(m, 0)
                for sbk in (1, 2, 3):
                    push_k_chain(m, sbk)

            # wo shares wv's pool slot; the DMA is emitted here but lands
            # late in the DMA order (v chains all emitted by then on PE).
            wo_sb = wvop.tile([P, MT, D], CDT, tag="wvo", name="wo_sb")
            nc.sync.dma_start(wo_sb[:], wo_d[:])

            def mk_rate(sb):
                def rate(i):
                    hp_step = i % NT
                    if sb == 0:
                        # deadline-matched dribble: V+K m0 in hp0, K m1-3
                        # and Q chains spread over hp1-3 (~2.5/step)
                        n = 10 if i < 16 else (3 if i % 2 == 0 else 2)
                        return (n, False)
                    # avoid stalling PE on the previous head-pair's norm
                    # chain (~5us): no lazy (y-chain) pulls until it is done.
                    return (2, hp_step >= 7)
                return rate

            otiles_by_sb = {sb: [] for sb in range(NSB)}
            for sb in range(NSB):
                if sb + 2 < NSB and (0, sb + 2) not in qtile:
                    # qT block sb+2 lands during sb, consumed during sb+1
                    load_block(qT_d, qtile, sb + 2, "qT")
                if sb + 1 < NSB:
                    for m in range(MT):
                        push_q_chain(m, sb + 1)
                emit_attn_sb(sb, otiles_by_sb[sb], mk_rate(sb))
                push_y_chains(sb, otiles_by_sb[sb])
            # keep the PE HAM warm through the final norm chain so the
            # drained output-projection matmuls run at 2.4GHz
            for i in range(8):
                ps_w = ps_pair.tile([P, SBW], F32, tag="psT", name="ps_tailwarm")
                nc.tensor.matmul(
                    ps_w[:, 0:P], wz[:, 0:P], wz[:, 0:P], start=True, stop=True
                )
            # drain whatever is left (final output projections)
            pull(len(work_q) + len(lazy_q))

    nc.compile()
    return nc


def _get_nc():
    global _NC
    if _NC is None:
        _install_ntff_shim()
        _NC = _build()
    return _NC


def _tile_blocks(xT):
    """[D, S] -> [KO, NSB, P, SBW] contiguous blocks."""
    return np.ascontiguousarray(
        xT.reshape(KO, P, NSB, SBW).transpose(0, 2, 1, 3)
    )


def make_in_maps(q, k, v, Wq, bq, Wk, bk, Wv, bv, Wo):
    """Shard + lay out the full inputs into the 8 per-core input maps."""
    in_maps = []
    for c in range(NCORES):
        b, j = divmod(c, 2)
        d0 = j * DP
        qT = np.ascontiguousarray(q[b].T).astype(BF16)
        kT = np.ascontiguousarray(k[b].T).astype(BF16)
        vT = np.ascontiguousarray(v[b].T).astype(BF16)
        wq = Wq[:, d0 : d0 + DP].astype(BF16)  # [D, DP]
        wk = Wk[:, d0 : d0 + DP].astype(BF16)
        wv = Wv[:, d0 : d0 + DP].astype(BF16)
        wo = Wo[d0 : d0 + DP, :].astype(BF16)  # [DP, D]
        # wq/wk m-sliced [MT, P, KO, P]: slice m covers out-cols m*128..,
        # [contraction-part p, ko, out-col] per slice.
        wq4 = np.ascontiguousarray(
            wq.reshape(KO, P, MT, P).transpose(2, 1, 0, 3)
        )
        wk4 = np.ascontiguousarray(
            wk.reshape(KO, P, MT, P).transpose(2, 1, 0, 3)
        )
        # wv [P, KO, DP]: [contraction-part, ko, out-col]
        wv3 = np.ascontiguousarray(wv.reshape(KO, P, DP).transpose(1, 0, 2))
        # wo [P, MT, D]: [contraction-part within hp, hp, out-col]
        wo3 = np.ascontiguousarray(wo.reshape(MT, P, D).transpose(1, 0, 2))
        in_maps.append(
            {
                "qT": _tile_blocks(qT),
                "kT": np.ascontiguousarray(
                    kT.reshape(KO, P, NSB // 2, 2 * SBW).transpose(0, 2, 1, 3)
                ),
                "vT": _tile_blocks(vT),
                "wq": wq4,
                "wk": wk4,
                "wv": wv3,
                "wo": wo3,
                "bq": np.ascontiguousarray(
                    bq[d0 : d0 + DP].reshape(MT, P).T
                ).astype(np.float32),
                "bk": np.ascontiguousarray(
                    bk[d0 : d0 + DP].reshape(MT, P).T
                ).astype(np.float32),
                "bv": bv[d0 : d0 + DP].reshape(1, DP).astype(BF16),
            }
        )
    return in_maps


def assemble_yT(blocks):
    """[KO, NSB, P, SBW] blocks -> [S, D] float32."""
    yT = np.asarray(blocks, dtype=np.float32)  # [KO, NSB, P, SBW]
    return yT.transpose(0, 2, 1, 3).reshape(D, S).T


def kernel(q, k, v, Wq, bq, Wk, bk, Wv, bv, Wo, bo, use_causal_mask=1):
    from concourse.bass_utils import run_bass_kernel_spmd

    q = np.asarray(q, np.float32)
    k = np.asarray(k, np.float32)
    v = np.asarray(v, np.float32)
    Wq = np.asarray(Wq, np.float32)
    Wk = np.asarray(Wk, np.float32)
    Wv = np.asarray(Wv, np.float32)
    Wo = np.asarray(Wo, np.float32)
    bq = np.asarray(bq, np.float32)
    bk = np.asarray(bk, np.float32)
    bv = np.asarray(bv, np.float32)
    bo = np.asarray(bo, np.float32)

    nc = _get_nc()
    in_maps = make_in_maps(q, k, v, Wq, bq, Wk, bk, Wv, bv, Wo)
    trace = bool(os.environ.get("KERNEL_TRACE"))
    res = run_bass_kernel_spmd(
        nc, in_maps, core_ids=list(range(NCORES)), trace=trace
    )
    LAST_RUN.clear()
    LAST_RUN.update(
        exec_time_ns=res.exec_time_ns,
        mean_exec_time_ns=res.mean_exec_time_ns,
        trace=(res.instructions_and_trace or (None, None))[1],
        per_core_scope_times=res.per_core_scope_times,
    )

    y = np.empty((B, S, D), np.float32)
    for b in range(B):
        acc = assemble_yT(res.results[2 * b]["yT"]) + assemble_yT(
            res.results[2 * b + 1]["yT"]
        )
        y[b] = acc + bo
    return y
